# revision 1
# baseline (speedup 1.0000x reference)
"""GCN classifier (2x GCNConv + mean-pool + 2-layer MLP) on 8 Trainium2 cores.

Sharding strategy (graph/data parallel per the hint):
- Nodes partitioned contiguously: core c owns dst nodes [c*6250, (c+1)*6250).
- conv1 (aggregate-then-transform): edges + self-loops partitioned by dst
  owner, grouped into 98 windows of 64 dst nodes, padded to 128-edge chunks
  (chunk counts maxed across cores -> one SPMD program). The host ships each
  core its incident edges' x rows pre-scaled by the full sym-norm
  dinv[src]*dinv[dst] and quantized to fp8-e4m3 (chunk-ordered -> one big
  sequential DMA stream per 512-node group). The scatter-add is realized on
  the PE as matmuls with the fp8 x chunk stationary and a 64-wide 0/1
  one-hot (bf16 iota-compare on DVE) as the moving operand, accumulating in
  PSUM -> the aggregation lands feature-major, no transposes. Dense W1
  (bf16) + bias + relu -> h1 kept feature-major in SBUF only.
- conv2 + mean-pool fused algebraically: with no nonlinearity between
  conv2's aggregation and the pooling, pooled sums satisfy
  pool[G] = sum_s A[s,G] * (h1[s] @ W2), where
  A[s,G] = dinv[s]*(sum_{e:src=s,dst in G} dinv[dst] + [batch[s]==G]*dinv[s])
  is built on host from edge_index/batch/deg only (structural data). Each
  core computes p = h1 @ W2 (bf16) for its own node chunks and immediately
  accumulates A_chunk^T @ p_chunk into a persistent [64,256] PSUM tile --
  no halo exchange, no gathers; p never leaves SBUF.
- Two pipelined 16KB fp8 AllReduces of the pooled partials: the first
  (node chunks 0-23) fires mid-kernel so its firmware launch and ring hide
  under compute; only the second, late piece sits on the critical path.
  mean+bias+relu and the tiny MLP run replicated; core 0's output wins.
- Pipelining: x_edges DMA + one-hot build per 512-node group (double
  buffered), aggregation/dense/p-chunks interleaved group by group so the
  PE stays busy (and the HAM clock stays warm) end to end.
"""

import sys
import types

import ml_dtypes
import numpy as np

try:
    import antenv  # noqa: F401

    if "antenv.axon_hooks" not in sys.modules:
        _m = types.ModuleType("antenv.axon_hooks")
        _m._hook = None
        _m.set_axon_ntff_profile_hook = lambda h: setattr(_m, "_hook", h)
        _m.get_axon_ntff_profile_hook = lambda: _m._hook
        sys.modules["antenv.axon_hooks"] = _m
except Exception:
    pass

import concourse.bacc as bacc
import concourse.mybir as mybir
import concourse.tile as tile
from concourse import bass_utils

F32 = mybir.dt.float32
BF16 = mybir.dt.bfloat16
F8 = mybir.dt.float8e4
AF = mybir.ActivationFunctionType
OP = mybir.AluOpType

N = 50000
E = 500000
DIN = 256
DH = 512
NG = 64
DOUT = 16

NCORES = 8
SLICE = N // NCORES  # 6250
WW = 64  # dst window width (one-hot width)
NW = (SLICE + WW - 1) // WW  # 98 windows
NPAD = 6272  # 49 * 128 node columns
NCHK = NPAD // 128  # 49 node chunks
NGRP = 13  # 12 groups of 512 node cols + 1 of 128

_COMPILED: dict = {}


def _group_info(g):
    """(first window, #windows, node col0, #node cols, first chunk, #chunks)"""
    if g < 12:
        return (8 * g, 8, 512 * g, 512, 4 * g, 4)
    return (96, 2, 6144, 128, 48, 1)


def _layout(K1):
    """Batches of <=4 windows: [(g, ws, {w: [(gcol, grel)]}, nch, c0)]."""
    batches = []
    gcol = 0
    for g in range(NGRP):
        w0, nwin, _, _, _, _ = _group_info(g)
        nhalf = 2 if nwin == 8 else 1
        for half in range(nhalf):
            ws = list(range(w0 + half * 4, min(w0 + (half + 1) * 4, w0 + nwin)))
            c0 = gcol
            rel = 0
            wch = {}
            for w in ws:
                lst = []
                for _ in range(int(K1[w])):
                    lst.append((gcol, rel))
                    gcol += 1
                    rel += 1
                wch[w] = lst
            batches.append((g, ws, wch, rel, c0))
    return batches, gcol


def _preprocess(x, edge_index, batch):
    src = np.asarray(edge_index[0], dtype=np.int64)
    dst = np.asarray(edge_index[1], dtype=np.int64)
    batch = np.asarray(batch, dtype=np.int64)

    deg = np.bincount(dst, minlength=N).astype(np.float64) + 1.0
    dinv = (1.0 / np.sqrt(deg)).astype(np.float32)
    cnt = np.maximum(np.bincount(batch, minlength=NG), 1)

    loops = np.arange(N, dtype=np.int64)

    # ---------- conv1: edges + self-loops grouped by (core, 64-window) ----------
    s1 = np.concatenate([src, loops])
    d1 = np.concatenate([dst, loops])
    norm1 = dinv[s1] * dinv[d1]
    core1 = d1 // SLICE
    win1 = (d1 % SLICE) // WW
    key1 = core1 * NW + win1
    order1 = np.argsort(key1, kind="stable")
    ss1, ds1, nn1 = s1[order1], d1[order1], norm1[order1]
    counts1 = np.bincount(key1, minlength=NCORES * NW).reshape(NCORES, NW)
    starts1 = np.zeros(NCORES * NW + 1, dtype=np.int64)
    np.cumsum(counts1.reshape(-1), out=starts1[1:])
    K1 = np.ceil(counts1.max(axis=0) / 128).astype(np.int64)  # [NW]

    meta = tuple(int(v) for v in K1)
    batches, C1 = _layout(K1)

    # ---------- fused conv2+pool coefficient matrix A[s, G] ----------
    gd = batch[dst]
    A = np.bincount(src * NG + gd, weights=dinv[dst].astype(np.float64),
                    minlength=N * NG).reshape(N, NG).astype(np.float32)
    A[loops, batch] += dinv
    A *= dinv[:, None]

    xf = np.asarray(x, np.float32)

    per_core = []
    for c in range(NCORES):
        src_cols = np.zeros((C1, 128), dtype=np.int64)
        nrm_cols = np.zeros((C1, 128), dtype=np.float32)
        dst_cols = np.full((C1, 128), -1.0, dtype=np.float32)
        for _g, ws, wch, _nch, _c0 in batches:
            for w in ws:
                gi = c * NW + w
                e0, e1 = starts1[gi], starts1[gi + 1]
                n_e = int(e1 - e0)
                cols = wch[w]
                k = len(cols)
                sv = np.zeros(k * 128, dtype=np.int64)
                sv[:n_e] = ss1[e0:e1]
                nv = np.zeros(k * 128, dtype=np.float32)
                nv[:n_e] = nn1[e0:e1]
                dv = np.full(k * 128, -1.0, dtype=np.float32)
                dv[:n_e] = (ds1[e0:e1] - (c * SLICE + w * WW)).astype(np.float32)
                for j, (gcol, _r) in enumerate(cols):
                    src_cols[gcol] = sv[j * 128 : (j + 1) * 128]
                    nrm_cols[gcol] = nv[j * 128 : (j + 1) * 128]
                    dst_cols[gcol] = dv[j * 128 : (j + 1) * 128]
        rows = xf[src_cols.reshape(-1)] * nrm_cols.reshape(-1)[:, None]
        x_edges = np.ascontiguousarray(
            rows.astype(ml_dtypes.float8_e4m3).reshape(C1, 128, DIN).transpose(1, 0, 2)
        ).reshape(128, C1 * DIN)

        Ac = np.zeros((NPAD, NG), dtype=np.float32)
        Ac[:SLICE] = A[c * SLICE : (c + 1) * SLICE]
        a_sb = np.ascontiguousarray(
            Ac.reshape(NCHK, 128, NG).transpose(1, 0, 2)
        ).reshape(128, NCHK * NG).astype(ml_dtypes.bfloat16)

        per_core.append(
            dict(
                x_edges=x_edges,
                dst1=np.ascontiguousarray(dst_cols.T).astype(ml_dtypes.bfloat16),
                a_mat=a_sb,
            )
        )
    return meta, per_core, cnt.astype(np.float32)


def _build_program(meta):
    K1 = np.array(meta)
    batches, C1 = _layout(K1)

    nc = bacc.Bacc("TRN2", target_bir_lowering=False, debug=False, num_devices=NCORES)

    def din(name, shape, dt=F32):
        return nc.dram_tensor(name, shape, dt, kind="ExternalInput").ap()

    x_edges = din("x_edges", [128, C1 * DIN], F8)
    dst1 = din("dst1", [128, C1], BF16)
    a_mat = din("a_mat", [128, NCHK * NG], BF16)
    iota64 = din("iota64", [128, WW], BF16)
    w1b = din("w1b", [128, 2 * DH], BF16)
    w2b = din("w2b", [128, 4 * (DH // 2)], BF16)
    b1c = din("b1c", [128, DH // 128])
    b2r = din("b2r", [128, DH // 2])
    wf1 = din("wf1", [128, 2 * (DH // 4)])
    bf1c = din("bf1c", [128, 1])
    wf2 = din("wf2", [DH // 4, DOUT])
    bf2c = din("bf2c", [DOUT, 1])
    cnt_in = din("cnt", [NG, 1])
    ident = din("ident", [128, 128])
    out = nc.dram_tensor("out", [DOUT, NG], F32, kind="ExternalOutput").ap()

    with tile.TileContext(nc) as tc:
        with (
            tc.tile_pool(name="const", bufs=1) as cp,
            tc.tile_pool(name="big", bufs=1) as bigp,
            tc.tile_pool(name="work", bufs=1) as wp,
            tc.tile_pool(name="psum", bufs=1, space="PSUM") as pp,
            tc.tile_pool(name="dram", bufs=1, space="DRAM") as dp,
        ):
            def load(ap_in, shape, dt=F32, pool=cp):
                t = pool.tile(shape, dt, name=ap_in.tensor.name + "_sb")
                nc.sync.dma_start(t[:], ap_in[:])
                return t

            # loads gating the pipeline start go first; the rest after batch 0
            dst1_sb = load(dst1, [128, C1], BF16)
            iota_sb = load(iota64, [128, WW], BF16)

            h1s = [bigp.tile([128, NPAD], BF16, name=f"h1s_{k}") for k in range(4)]

            sfm_groups: dict = {}

            def sfm_of(g):
                if g not in sfm_groups:
                    sfm_groups[g] = [
                        wp.tile([128, 512], BF16, tag=f"sfm{h}", bufs=2, name=f"sfm{h}_{g}")
                        for h in range(2)
                    ]
                return sfm_groups[g]

            CCSPLIT = 24  # pgx_a: chunks [0,24) reduced early; pgx_b: the rest
            pgx = pp.tile([NG, DH // 2], F32, name="pgx")
            pgx_b = pp.tile([NG, DH // 2], F32, tag="t", bufs=1, name="pgx_b")
            g_local = [dp.tile([NG, DH // 2], F8, name=f"gl{i}") for i in range(2)]
            g_red = [
                dp.tile([NG, DH // 2], F8, addr_space="Shared", name=f"gr{i}")
                for i in range(2)
            ]

            gs2 = [wp.tile([NG, DH // 2], F8, name=f"gs{i}") for i in range(2)]

            def emit_allreduce(i):
                src = pgx if i == 0 else pgx_b
                gsb = wp.tile([NG, DH // 2], F8, name=f"gsb{i}")
                nc.vector.tensor_copy(gsb[:], src[:])
                nc.sync.dma_start(g_local[i][:], gsb[:])
                nc.gpsimd.collective_compute(
                    "AllReduce",
                    OP.add,
                    replica_groups=[list(range(NCORES))],
                    ins=[g_local[i].opt()],
                    outs=[g_red[i].opt()],
                )

            def emit_stream(batchpair):
                """One G1 DMA + one-hot build covering a whole group's batches."""
                c0 = batchpair[0][4]
                nch = sum(b[3] for b in batchpair)
                g = batchpair[0][0]
                G1 = wp.tile([128, nch, DIN], F8, tag="G1", bufs=2, name=f"g1g_{g}")
                nc.sync.dma_start(
                    G1[:].rearrange("p c d -> p (c d)"),
                    x_edges[:, c0 * DIN : (c0 + nch) * DIN],
                )
                oh = wp.tile([128, nch, WW], BF16, tag="oh", bufs=2, name=f"ohg_{g}")
                nc.vector.tensor_tensor(
                    out=oh[:],
                    in0=iota_sb[:].rearrange("p (o i) -> p o i", o=1).to_broadcast([128, nch, WW]),
                    in1=dst1_sb[:, c0 : c0 + nch].rearrange("p (c o) -> p c o", o=1).to_broadcast([128, nch, WW]),
                    op=OP.is_equal,
                )
                return G1, oh, c0

            def emit_batch(g, ws, wch, nch, c0, G1, oh, gc0):
                rel0 = c0 - gc0
                sf = sfm_of(g)
                nw = len(ws)
                wb0 = ws[0] - 8 * g
                pa = pp.tile([128, nw, 2, WW], F32, tag="agg", bufs=2, name=f"pa_{ws[0]}")
                for w in ws:
                    cols = wch[w]
                    wrel = w - ws[0]
                    for j, (_gcol, grel) in enumerate(cols):
                        for h in range(2):
                            nc.tensor.matmul(
                                out=pa[:, wrel, h, :],
                                lhsT=G1[:, rel0 + grel, h * 128 : (h + 1) * 128],
                                rhs=oh[:, rel0 + grel, :],
                                start=(j == 0),
                                stop=(j == len(cols) - 1),
                            )
                for h in range(2):
                    nc.scalar.activation(
                        sf[h][:, wb0 * WW : (wb0 + nw) * WW],
                        pa[:, :, h, :],
                        AF.Copy,
                    )

            def emit_dense(g):
                _, _, n0, ncols, _, _ = _group_info(g)
                sf = sfm_of(g)
                for m in range(4):
                    ph = pp.tile([128, 512], F32, tag="h1", bufs=2, name=f"ph_{g}_{m}")
                    for k in range(2):
                        nc.tensor.matmul(
                            out=ph[:, :ncols],
                            lhsT=w1_sb[:, k * DH + m * 128 : k * DH + (m + 1) * 128],
                            rhs=sf[k][:, :ncols],
                            start=(k == 0),
                            stop=(k == 1),
                        )
                    nc.scalar.activation(
                        h1s[m][:, n0 : n0 + ncols], ph[:, :ncols], AF.Relu,
                        bias=b1_sb[:, m : m + 1],
                    )

            def emit_pA(cc):
                c0 = cc * 128
                ppm = pp.tile([128, DH // 2], F32, tag="p2", bufs=2, name=f"ppm_{cc}")
                for k in range(4):
                    nc.tensor.matmul(
                        out=ppm[:],
                        lhsT=h1s[k][:, c0 : c0 + 128],
                        rhs=w2_sb[:, k * (DH // 2) : (k + 1) * (DH // 2)],
                        start=(k == 0),
                        stop=(k == 3),
                    )
                pb = wp.tile([128, DH // 2], BF16, tag="pb", bufs=2, name=f"pb_{cc}")
                nc.vector.tensor_copy(pb[:], ppm[:])
                nc.tensor.matmul(
                    out=(pgx if cc < CCSPLIT else pgx_b)[:],
                    lhsT=a_sb[:, cc * NG : (cc + 1) * NG],
                    rhs=pb[:],
                    start=(cc in (0, CCSPLIT)),
                    stop=(cc in (CCSPLIT - 1, NCHK - 1)),
                )

            bidx = 0
            streams = {0: emit_stream(batches[0:2]), 1: emit_stream(batches[2:4])}
            w1_sb = load(w1b, [128, 2 * DH], BF16)
            b1_sb = load(b1c, [128, DH // 128])
            for g in range(NGRP):
                _, nwin, _, _, cc0, nccs = _group_info(g)
                nb = 2 if nwin == 8 else 1
                bp = batches[bidx : bidx + nb]
                G1, oh, gc0 = streams.pop(g) if g in streams else emit_stream(bp)
                for b in bp:
                    emit_batch(*b, G1, oh, gc0)
                    bidx += 1
                if g == 0:
                    a_sb = load(a_mat, [128, NCHK * NG], BF16)
                    w2_sb = load(w2b, [128, 4 * (DH // 2)], BF16)
                    b2_sb = load(b2r, [128, DH // 2])
                    wf1_sb = load(wf1, [128, 2 * (DH // 4)])
                    bf1_sb = load(bf1c, [128, 1])
                    wf2_sb = load(wf2, [DH // 4, DOUT])
                    bf2_sb = load(bf2c, [DOUT, 1])
                    cnt_sb = load(cnt_in, [NG, 1])
                    idf32 = load(ident, [128, 128])
                emit_dense(g)
                for cc in range(cc0, cc0 + nccs):
                    emit_pA(cc)
                    if cc == CCSPLIT - 1:
                        emit_allreduce(0)
                        nc.sync.dma_start(gs2[0][:], g_red[0][:])

            # -------- tail: second AllReduce (small late piece) + combine --------
            emit_allreduce(1)
            nc.sync.dma_start(gs2[1][:], g_red[1][:])
            cur = wp.tile([NG, DH // 2], F32, name="gsum")
            nc.vector.tensor_tensor(out=cur[:], in0=gs2[0][:], in1=gs2[1][:], op=OP.add)

            cinv = wp.tile([NG, 1], F32)
            nc.vector.reciprocal(cinv[:], cnt_sb[:])
            gmean = wp.tile([NG, DH // 2], F32)
            nc.vector.scalar_tensor_tensor(
                out=gmean[:],
                in0=cur[:],
                scalar=cinv[:, 0:1],
                in1=b2_sb[:NG, :],
                op0=OP.mult,
                op1=OP.add,
            )
            grelu = wp.tile([NG, DH // 2], F32)
            nc.scalar.activation(grelu[:], gmean[:], AF.Relu)

            g_fm = [wp.tile([128, NG], F32, name=f"gfm_{k}") for k in range(2)]
            for k in range(2):
                pt = pp.tile([128, NG], F32, tag="t", bufs=1, name=f"gt_{k}")
                nc.tensor.transpose(pt[:], grelu[:, k * 128 : (k + 1) * 128], idf32[:NG, :NG])
                nc.vector.tensor_copy(g_fm[k][:], pt[:])
            pz = pp.tile([128, NG], F32, tag="h1", bufs=2, name="pz")
            for k in range(2):
                nc.tensor.matmul(
                    out=pz[:],
                    lhsT=wf1_sb[:, k * 128 : (k + 1) * 128],
                    rhs=g_fm[k][:],
                    start=(k == 0),
                    stop=(k == 1),
                )
            zsb = wp.tile([128, NG], F32)
            nc.scalar.activation(zsb[:], pz[:], AF.Relu, bias=bf1_sb[:, 0:1])
            po = pp.tile([DOUT, NG], F32, tag="t", bufs=1, name="po")
            nc.tensor.matmul(out=po[:], lhsT=wf2_sb[:], rhs=zsb[:], start=True, stop=True)
            osb = wp.tile([DOUT, NG], F32)
            nc.scalar.activation(osb[:], po[:], AF.Relu, bias=bf2_sb[:, 0:1])
            nc.sync.dma_start(out[:], osb[:])

    nc.compile()
    return nc


def _get_program(meta):
    if meta not in _COMPILED:
        _COMPILED[meta] = _build_program(meta)
    return _COMPILED[meta]


def _make_in_maps(W1, b1, W2, b2, Wf1, bf1, Wf2, bf2, per_core, cnt):
    bf = ml_dtypes.bfloat16
    W1 = np.asarray(W1, np.float32)
    W2 = np.asarray(W2, np.float32)
    Wf1 = np.asarray(Wf1, np.float32)
    shared = dict(
        iota64=np.tile(np.arange(WW, dtype=np.float32)[None, :], (128, 1)).astype(bf),
        w1b=np.ascontiguousarray(
            np.concatenate([W1[0:128, :], W1[128:256, :]], axis=1)
        ).astype(bf),
        w2b=np.ascontiguousarray(
            np.concatenate([W2[k * 128 : (k + 1) * 128, :] for k in range(4)], axis=1)
        ).astype(bf),
        b1c=np.ascontiguousarray(np.asarray(b1, np.float32).reshape(DH // 128, 128).T),
        b2r=np.ascontiguousarray(np.tile(np.asarray(b2, np.float32)[None, :], (128, 1))),
        wf1=np.ascontiguousarray(np.concatenate([Wf1[0:128, :], Wf1[128:256, :]], axis=1)),
        bf1c=np.tile(np.asarray(bf1, np.float32).reshape(DH // 4, 1), (1, 1)),
        wf2=np.asarray(Wf2, np.float32),
        bf2c=np.asarray(bf2, np.float32).reshape(DOUT, 1),
        cnt=np.asarray(cnt, np.float32).reshape(NG, 1),
        ident=np.eye(128, dtype=np.float32),
    )
    return [dict(shared, **per_core[c]) for c in range(NCORES)]


def kernel(
    x, W1, b1, W2, b2, Wf1, bf1, Wf2, bf2, edge_index, batch, num_graphs, _trace=False
):
    assert int(num_graphs) == NG
    meta, per_core, cnt = _preprocess(
        np.asarray(x), np.asarray(edge_index), np.asarray(batch)
    )
    nc = _get_program(meta)
    in_maps = _make_in_maps(W1, b1, W2, b2, Wf1, bf1, Wf2, bf2, per_core, cnt)
    res = bass_utils.run_bass_kernel_spmd(
        nc, in_maps, core_ids=list(range(NCORES)), trace=_trace
    )
    out = np.ascontiguousarray(np.asarray(res.results[0]["out"], np.float32).T)
    if _trace:
        kernel._last_results = res
    return out



# revision 6
# speedup vs baseline: 1.0631x; 1.0631x over previous
"""GCN classifier (2x GCNConv + mean-pool + 2-layer MLP) on 8 Trainium2 cores.

Sharding strategy (graph/data parallel per the hint):
- Nodes partitioned contiguously: core c owns dst nodes [c*6250, (c+1)*6250).
- conv1 (aggregate-then-transform): edges + self-loops partitioned by dst
  owner, grouped into 98 windows of 64 dst nodes, padded to 128-edge chunks
  (chunk counts maxed across cores -> one SPMD program). The host ships each
  core its incident edges' x rows pre-scaled by the full sym-norm
  dinv[src]*dinv[dst] and quantized to fp8-e4m3 (chunk-ordered -> one big
  sequential DMA stream per batch of <=4 windows). The scatter-add is
  realized on the PE as matmuls with the fp8 x chunk stationary (FWL) and a
  64-wide 0/1 one-hot (iota-compare on DVE/Pool) as the moving operand,
  accumulating in PSUM -> the aggregation lands feature-major, no
  transposes. Dense W1 applied with fp8 DoubleRow matmuls (both 128-row
  k-tiles in one pass, W1 stationary) + bias + relu -> h1 kept feature-major
  in SBUF only (bf16).
- conv2 + mean-pool fused algebraically: with no nonlinearity between
  conv2's aggregation and the pooling, pooled sums satisfy
  pool[G] = sum_s A[s,G] * (h1[s] @ W2), where
  A[s,G] = dinv[s]*(sum_{e:src=s,dst in G} dinv[dst] + [batch[s]==G]*dinv[s])
  is built on host from edge_index/batch/deg only (structural data). Each
  core computes p = h1 @ W2 (bf16) for its own node chunks and immediately
  accumulates pb^T @ A_chunk into persistent [128,64] PSUM tiles, keeping
  the pooled partials FEATURE-major -- the tail MLP then needs no
  transposes at all.
- One 16KB fp8 AllReduce of the pooled partials at the end (two ARs
  serialize on the CC stream, so a single late one exposes less latency).
  mean+bias+relu and the tiny MLP run replicated in feature-major layout
  (out lands as [DOUT, NG] directly); core 0's output wins.
- Pipelining: per-batch x_edges DMA + one-hot build (triple buffered,
  issue-ahead 2), aggregation/dense/p-chunks interleaved batch by batch so
  the PE stays busy end to end.
"""

import sys
import types

import ml_dtypes
import numpy as np

try:
    import antenv  # noqa: F401

    if "antenv.axon_hooks" not in sys.modules:
        _m = types.ModuleType("antenv.axon_hooks")
        _m._hook = None
        _m.set_axon_ntff_profile_hook = lambda h: setattr(_m, "_hook", h)
        _m.get_axon_ntff_profile_hook = lambda: _m._hook
        sys.modules["antenv.axon_hooks"] = _m
except Exception:
    pass

import concourse.bacc as bacc
import concourse.mybir as mybir
import concourse.tile as tile
from concourse import bass_utils

F32 = mybir.dt.float32
BF16 = mybir.dt.bfloat16
F8 = mybir.dt.float8e4
AF = mybir.ActivationFunctionType
OP = mybir.AluOpType
DR = mybir.MatmulPerfMode.DoubleRow

N = 50000
E = 500000
DIN = 256
DH = 512
NG = 64
DOUT = 16

NCORES = 8
SLICE = N // NCORES  # 6250
WW = 64  # dst window width (one-hot width)
NW = (SLICE + WW - 1) // WW  # 98 windows
NPAD = 6272  # 49 * 128 node columns
NCHK = NPAD // 128  # 49 node chunks
NGRP = 13  # 12 groups of 512 node cols + 1 of 128

# tuning knobs
USE_DR_DENSE = True  # fp8 DoubleRow for the W1 dense
OH_ON_GPSIMD = False  # Pool engine lacks is_equal (walrus ISA check)
N_AR = 1  # number of pool AllReduce pieces (1 or 2)
AR_SPLIT = 24  # first-AR chunk split when N_AR == 2

_COMPILED: dict = {}


def _group_info(g):
    """(first window, #windows, node col0, #node cols, first chunk, #chunks)"""
    if g < 12:
        return (8 * g, 8, 512 * g, 512, 4 * g, 4)
    return (96, 2, 6144, 128, 48, 1)


def _layout(K1):
    """Batches of <=4 windows: [(g, ws, {w: [(gcol, grel)]}, nch, c0)]."""
    batches = []
    gcol = 0
    for g in range(NGRP):
        w0, nwin, _, _, _, _ = _group_info(g)
        nhalf = 2 if nwin == 8 else 1
        for half in range(nhalf):
            ws = list(range(w0 + half * 4, min(w0 + (half + 1) * 4, w0 + nwin)))
            c0 = gcol
            rel = 0
            wch = {}
            for w in ws:
                lst = []
                for _ in range(int(K1[w])):
                    lst.append((gcol, rel))
                    gcol += 1
                    rel += 1
                wch[w] = lst
            batches.append((g, ws, wch, rel, c0))
    return batches, gcol


def _preprocess(x, edge_index, batch):
    src = np.asarray(edge_index[0], dtype=np.int64)
    dst = np.asarray(edge_index[1], dtype=np.int64)
    batch = np.asarray(batch, dtype=np.int64)

    deg = np.bincount(dst, minlength=N).astype(np.float64) + 1.0
    dinv = (1.0 / np.sqrt(deg)).astype(np.float32)
    cnt = np.maximum(np.bincount(batch, minlength=NG), 1)

    loops = np.arange(N, dtype=np.int64)

    # ---------- conv1: edges + self-loops grouped by (core, 64-window) ----------
    s1 = np.concatenate([src, loops])
    d1 = np.concatenate([dst, loops])
    norm1 = dinv[s1] * dinv[d1]
    core1 = d1 // SLICE
    win1 = (d1 % SLICE) // WW
    key1 = core1 * NW + win1
    order1 = np.argsort(key1, kind="stable")
    ss1, ds1, nn1 = s1[order1], d1[order1], norm1[order1]
    counts1 = np.bincount(key1, minlength=NCORES * NW).reshape(NCORES, NW)
    starts1 = np.zeros(NCORES * NW + 1, dtype=np.int64)
    np.cumsum(counts1.reshape(-1), out=starts1[1:])
    K1 = np.ceil(counts1.max(axis=0) / 128).astype(np.int64)  # [NW]

    meta = tuple(int(v) for v in K1)
    batches, C1 = _layout(K1)

    # ---------- fused conv2+pool coefficient matrix A[s, G] ----------
    gd = batch[dst]
    A = np.bincount(src * NG + gd, weights=dinv[dst].astype(np.float64),
                    minlength=N * NG).reshape(N, NG).astype(np.float32)
    A[loops, batch] += dinv
    A *= dinv[:, None]

    xf = np.asarray(x, np.float32)

    per_core = []
    for c in range(NCORES):
        src_cols = np.zeros((C1, 128), dtype=np.int64)
        nrm_cols = np.zeros((C1, 128), dtype=np.float32)
        dst_cols = np.full((C1, 128), -1.0, dtype=np.float32)
        for _g, ws, wch, _nch, _c0 in batches:
            for w in ws:
                gi = c * NW + w
                e0, e1 = starts1[gi], starts1[gi + 1]
                n_e = int(e1 - e0)
                cols = wch[w]
                k = len(cols)
                sv = np.zeros(k * 128, dtype=np.int64)
                sv[:n_e] = ss1[e0:e1]
                nv = np.zeros(k * 128, dtype=np.float32)
                nv[:n_e] = nn1[e0:e1]
                dv = np.full(k * 128, -1.0, dtype=np.float32)
                dv[:n_e] = (ds1[e0:e1] - (c * SLICE + w * WW)).astype(np.float32)
                for j, (gcol, _r) in enumerate(cols):
                    src_cols[gcol] = sv[j * 128 : (j + 1) * 128]
                    nrm_cols[gcol] = nv[j * 128 : (j + 1) * 128]
                    dst_cols[gcol] = dv[j * 128 : (j + 1) * 128]
        rows = xf[src_cols.reshape(-1)] * nrm_cols.reshape(-1)[:, None]
        x_edges = np.ascontiguousarray(
            rows.astype(ml_dtypes.float8_e4m3).reshape(C1, 128, DIN).transpose(1, 0, 2)
        ).reshape(128, C1 * DIN)

        Ac = np.zeros((NPAD, NG), dtype=np.float32)
        Ac[:SLICE] = A[c * SLICE : (c + 1) * SLICE]
        a_sb = np.ascontiguousarray(
            Ac.reshape(NCHK, 128, NG).transpose(1, 0, 2)
        ).reshape(128, NCHK * NG).astype(ml_dtypes.bfloat16)

        per_core.append(
            dict(
                x_edges=x_edges,
                dst1=np.ascontiguousarray(dst_cols.T).astype(ml_dtypes.bfloat16),
                a_mat=a_sb,
            )
        )
    return meta, per_core, cnt.astype(np.float32)


def _build_program(meta):
    K1 = np.array(meta)
    batches, C1 = _layout(K1)
    max_nch = max(b[3] for b in batches)

    nc = bacc.Bacc("TRN2", target_bir_lowering=False, debug=False, num_devices=NCORES)

    def din(name, shape, dt=F32):
        return nc.dram_tensor(name, shape, dt, kind="ExternalInput").ap()

    x_edges = din("x_edges", [128, C1 * DIN], F8)
    dst1 = din("dst1", [128, C1], BF16)
    a_mat = din("a_mat", [128, NCHK * NG], BF16)
    iota_rep = din("iota_rep", [128, max_nch * WW], BF16)
    w1dr = din("w1dr", [128, 2 * DH], F8)  # [p, k(2), m(4), 128] fp8 pairs
    w1bf = din("w1bf", [128, 2 * DH], BF16)  # [p, k(2), fo(512)] bf16 fallback
    w2b = din("w2b", [128, 4 * (DH // 2)], BF16)
    b1c = din("b1c", [128, DH // 128])
    b2h = din("b2h", [128, 2])  # b2 feature-major halves
    cinv2 = din("cinv2", [128, 2 * NG])  # 1/cnt tiled [p, h, G]
    wf1 = din("wf1", [128, 2 * (DH // 4)], BF16)  # [p, k(2), 128] bf16
    bf1c = din("bf1c", [128, 1])
    wf2 = din("wf2", [DH // 4, DOUT], BF16)
    bf2c = din("bf2c", [DOUT, 1])
    out = nc.dram_tensor("out", [DOUT, NG], F32, kind="ExternalOutput").ap()

    with tile.TileContext(nc) as tc:
        with (
            tc.tile_pool(name="const", bufs=1) as cp,
            tc.tile_pool(name="big", bufs=1) as bigp,
            tc.tile_pool(name="work", bufs=1) as wp,
            tc.tile_pool(name="psum", bufs=1, space="PSUM") as pp,
            tc.tile_pool(name="dram", bufs=1, space="DRAM") as dp,
        ):
            def load(ap_in, shape, dt=F32, pool=cp):
                t = pool.tile(shape, dt, name=ap_in.tensor.name + "_sb")
                nc.sync.dma_start(t[:], ap_in[:])
                return t

            # loads gating the pipeline start go first; the rest after batch 0
            dst1_sb = load(dst1, [128, C1], BF16)
            iota_sb = load(iota_rep, [128, max_nch * WW], BF16)

            h1s = [bigp.tile([128, NPAD], BF16, name=f"h1s_{k}") for k in range(4)]

            sfg_groups: dict = {}

            def sfg_of(g):
                # fp8 feature-major conv1 aggregation for group g: [p, k(2), n]
                if g not in sfg_groups:
                    sfg_groups[g] = wp.tile(
                        [128, 2, 512], F8, tag="sfg", bufs=2, name=f"sfg_{g}"
                    )
                return sfg_groups[g]

            # persistent feature-major pool partials: pgo[i][h] = [128 o, 64 G]
            pgo = [
                [
                    pp.tile([128, NG], F32, name=f"pgo_{i}_{h}")
                    for h in range(2)
                ]
                for i in range(N_AR)
            ]
            g_local = [dp.tile([128, 2 * NG], F8, name=f"gl{i}") for i in range(N_AR)]
            g_red = [
                dp.tile([128, 2 * NG], F8, addr_space="Shared", name=f"gr{i}")
                for i in range(N_AR)
            ]
            gs2 = [wp.tile([128, 2 * NG], F8, name=f"gs{i}") for i in range(N_AR)]

            def emit_allreduce(i):
                gsb = wp.tile([128, 2, NG], F8, name=f"gsb{i}")
                for h in range(2):
                    nc.vector.tensor_copy(gsb[:, h, :], pgo[i][h][:])
                nc.sync.dma_start(g_local[i][:], gsb[:].rearrange("p h g -> p (h g)"))
                nc.gpsimd.collective_compute(
                    "AllReduce",
                    OP.add,
                    replica_groups=[list(range(NCORES))],
                    ins=[g_local[i].opt()],
                    outs=[g_red[i].opt()],
                )
                nc.sync.dma_start(gs2[i][:], g_red[i][:])

            def emit_stream(bi):
                """One G1 DMA + one-hot build covering one batch (<=4 windows)."""
                _g, _ws, _wch, nch, c0 = batches[bi]
                G1 = wp.tile([128, nch, DIN], F8, tag="G1", bufs=3, name=f"g1b_{bi}")
                nc.sync.dma_start(
                    G1[:].rearrange("p c d -> p (c d)"),
                    x_edges[:, c0 * DIN : (c0 + nch) * DIN],
                )
                oh = wp.tile([128, nch, WW], BF16, tag="oh", bufs=3, name=f"ohb_{bi}")
                eng = nc.gpsimd if (OH_ON_GPSIMD and bi % 2 == 1) else nc.vector
                eng.tensor_tensor(
                    out=oh[:],
                    in0=iota_sb[:, : nch * WW].rearrange("p (c o) -> p c o", o=WW),
                    in1=dst1_sb[:, c0 : c0 + nch]
                    .rearrange("p (c o) -> p c o", o=1)
                    .to_broadcast([128, nch, WW]),
                    op=OP.is_equal,
                )
                return G1, oh

            def emit_batch(g, ws, wch, nch, c0, G1, oh):
                sfg = sfg_of(g)
                nw = len(ws)
                wb0 = ws[0] - 8 * g
                pa = pp.tile([128, nw, 2, WW], F32, tag="agg", bufs=2, name=f"pa_{ws[0]}")
                for w in ws:
                    cols = wch[w]
                    wrel = w - ws[0]
                    for j, (_gcol, grel) in enumerate(cols):
                        for h in range(2):
                            nc.tensor.matmul(
                                out=pa[:, wrel, h, :],
                                lhsT=G1[:, grel, h * 128 : (h + 1) * 128],
                                rhs=oh[:, grel, :],
                                start=(j == 0),
                                stop=(j == len(cols) - 1),
                            )
                for h in range(2):
                    nc.vector.tensor_copy(
                        sfg[:, h, wb0 * WW : (wb0 + nw) * WW],
                        pa[:, :, h, :],
                    )

            def emit_dense(g):
                _, _, n0, ncols, _, _ = _group_info(g)
                sfg = sfg_of(g)
                for m in range(4):
                    ph = pp.tile([128, 512], F32, tag="h1", bufs=2, name=f"ph_{g}_{m}")
                    if USE_DR_DENSE:
                        nc.tensor.matmul(
                            out=ph[:, :ncols],
                            lhsT=w1_sb[:, :, m, :],
                            rhs=sfg[:, :, :ncols],
                            start=True,
                            stop=True,
                            perf_mode=DR,
                        )
                    else:
                        for k in range(2):
                            nc.tensor.matmul(
                                out=ph[:, :ncols],
                                lhsT=w1f_sb[:, k, m * 128 : (m + 1) * 128],
                                rhs=sfg[:, k, :ncols],
                                start=(k == 0),
                                stop=(k == 1),
                            )
                    nc.scalar.activation(
                        h1s[m][:, n0 : n0 + ncols], ph[:, :ncols], AF.Relu,
                        bias=b1_sb[:, m : m + 1],
                    )

            def emit_pA(cc):
                c0 = cc * 128
                ppm = pp.tile([128, DH // 2], F32, tag="p2", bufs=2, name=f"ppm_{cc}")
                for k in range(4):
                    nc.tensor.matmul(
                        out=ppm[:],
                        lhsT=h1s[k][:, c0 : c0 + 128],
                        rhs=w2_sb[:, k * (DH // 2) : (k + 1) * (DH // 2)],
                        start=(k == 0),
                        stop=(k == 3),
                    )
                pb = wp.tile([128, DH // 2], BF16, tag="pb", bufs=2, name=f"pb_{cc}")
                nc.scalar.activation(pb[:], ppm[:], AF.Copy)
                ar = 0 if (N_AR == 1 or cc < AR_SPLIT) else 1
                first = cc == 0 or (ar == 1 and cc == AR_SPLIT)
                last = cc == NCHK - 1 or (ar == 0 and N_AR == 2 and cc == AR_SPLIT - 1)
                for h in range(2):
                    nc.tensor.matmul(
                        out=pgo[ar][h][:],
                        lhsT=pb[:, h * 128 : (h + 1) * 128],
                        rhs=a_sb[:, cc * NG : (cc + 1) * NG],
                        start=first,
                        stop=last,
                    )

            streams = {0: emit_stream(0), 1: emit_stream(1)}
            if USE_DR_DENSE:
                w1_sb = load(w1dr, [128, 2, 4, 128], F8)
            else:
                w1f_sb = load(w1bf, [128, 2, DH], BF16)
            b1_sb = load(b1c, [128, DH // 128])
            bidx = 0
            for g in range(NGRP):
                _, nwin, _, _, cc0, nccs = _group_info(g)
                nb = 2 if nwin == 8 else 1
                for _b in range(nb):
                    if bidx + 2 < len(batches):
                        streams[bidx + 2] = emit_stream(bidx + 2)
                    G1, oh = streams.pop(bidx)
                    emit_batch(*batches[bidx], G1, oh)
                    bidx += 1
                if g == 0:
                    a_sb = load(a_mat, [128, NCHK * NG], BF16)
                    w2_sb = load(w2b, [128, 4 * (DH // 2)], BF16)
                    b2_sb = load(b2h, [128, 2])
                    ci_sb = load(cinv2, [128, 2 * NG])
                    wf1_sb = load(wf1, [128, 2, DH // 4], BF16)
                    bf1_sb = load(bf1c, [128, 1])
                    wf2_sb = load(wf2, [DH // 4, DOUT], BF16)
                    bf2_sb = load(bf2c, [DOUT, 1])
                emit_dense(g)
                for cc in range(cc0, cc0 + nccs):
                    emit_pA(cc)
                    if N_AR == 2 and cc == AR_SPLIT - 1:
                        emit_allreduce(0)

            # -------- tail: AllReduce + mean/bias/relu + MLP, all feature-major
            emit_allreduce(N_AR - 1)
            cur = wp.tile([128, 2 * NG], F32, name="gsum")
            if N_AR == 2:
                nc.vector.tensor_tensor(
                    out=cur[:], in0=gs2[0][:], in1=gs2[1][:], op=OP.add
                )
                nc.vector.tensor_tensor(
                    out=cur[:], in0=cur[:], in1=ci_sb[:], op=OP.mult
                )
            else:
                nc.vector.tensor_tensor(
                    out=cur[:], in0=gs2[0][:], in1=ci_sb[:], op=OP.mult
                )
            grelu = wp.tile([128, 2, NG], BF16, name="grelu")
            curv = cur[:].rearrange("p (h g) -> p h g", h=2)
            for h in range(2):
                nc.scalar.activation(
                    grelu[:, h, :], curv[:, h, :], AF.Relu, bias=b2_sb[:, h : h + 1]
                )
            pz = pp.tile([128, NG], F32, tag="p2", bufs=2, name="pz")
            for k in range(2):
                nc.tensor.matmul(
                    out=pz[:],
                    lhsT=wf1_sb[:, k, :],
                    rhs=grelu[:, k, :],
                    start=(k == 0),
                    stop=(k == 1),
                )
            zsb = wp.tile([128, NG], BF16, name="zsb")
            nc.scalar.activation(zsb[:], pz[:], AF.Relu, bias=bf1_sb[:, 0:1])
            po = pp.tile([DOUT, NG], F32, tag="agg", bufs=2, name="po")
            nc.tensor.matmul(out=po[:], lhsT=wf2_sb[:], rhs=zsb[:], start=True, stop=True)
            osb = wp.tile([DOUT, NG], F32, name="osb")
            nc.scalar.activation(osb[:], po[:], AF.Relu, bias=bf2_sb[:, 0:1])
            nc.sync.dma_start(out[:], osb[:])

    nc.compile()
    return nc


def _get_program(meta):
    if meta not in _COMPILED:
        _COMPILED[meta] = _build_program(meta)
    return _COMPILED[meta]


def _make_in_maps(W1, b1, W2, b2, Wf1, bf1, Wf2, bf2, per_core, cnt, meta):
    bf = ml_dtypes.bfloat16
    f8 = ml_dtypes.float8_e4m3
    W1 = np.asarray(W1, np.float32)
    W2 = np.asarray(W2, np.float32)
    Wf1 = np.asarray(Wf1, np.float32)
    b2 = np.asarray(b2, np.float32)
    K1 = np.array(meta)
    batches, _C1 = _layout(K1)
    max_nch = max(b[3] for b in batches)

    # w1dr[p, k, m, c] = W1[k*128+p, m*128+c]
    w1dr = np.ascontiguousarray(
        W1.reshape(2, 128, 4, 128).transpose(1, 0, 2, 3).reshape(128, 2 * DH)
    )
    w1bf = np.ascontiguousarray(
        W1.reshape(2, 128, DH).transpose(1, 0, 2).reshape(128, 2 * DH)
    )
    cinv = (1.0 / np.asarray(cnt, np.float32)).reshape(1, NG)
    shared = dict(
        iota_rep=np.ascontiguousarray(
            np.tile(np.arange(WW, dtype=np.float32)[None, None, :],
                    (128, max_nch, 1)).reshape(128, max_nch * WW)
        ).astype(bf),
        w1dr=w1dr.astype(f8),
        w1bf=w1bf.astype(bf),
        w2b=np.ascontiguousarray(
            np.concatenate([W2[k * 128 : (k + 1) * 128, :] for k in range(4)], axis=1)
        ).astype(bf),
        b1c=np.ascontiguousarray(np.asarray(b1, np.float32).reshape(DH // 128, 128).T),
        b2h=np.ascontiguousarray(b2.reshape(2, 128).T),
        cinv2=np.ascontiguousarray(np.tile(cinv, (128, 2))),
        wf1=np.ascontiguousarray(
            Wf1.reshape(2, 128, DH // 4).transpose(1, 0, 2).reshape(128, 2 * (DH // 4))
        ).astype(bf),
        bf1c=np.tile(np.asarray(bf1, np.float32).reshape(DH // 4, 1), (1, 1)),
        wf2=np.asarray(Wf2, np.float32).astype(bf),
        bf2c=np.asarray(bf2, np.float32).reshape(DOUT, 1),
    )
    return [dict(shared, **per_core[c]) for c in range(NCORES)]


def kernel(
    x, W1, b1, W2, b2, Wf1, bf1, Wf2, bf2, edge_index, batch, num_graphs, _trace=False
):
    assert int(num_graphs) == NG
    meta, per_core, cnt = _preprocess(
        np.asarray(x), np.asarray(edge_index), np.asarray(batch)
    )
    nc = _get_program(meta)
    in_maps = _make_in_maps(W1, b1, W2, b2, Wf1, bf1, Wf2, bf2, per_core, cnt, meta)
    res = bass_utils.run_bass_kernel_spmd(
        nc, in_maps, core_ids=list(range(NCORES)), trace=_trace
    )
    out = np.ascontiguousarray(np.asarray(res.results[0]["out"], np.float32).T)
    if _trace:
        kernel._last_results = res
    return out


# revision 12
# speedup vs baseline: 1.1288x; 1.0619x over previous
"""GCN classifier (2x GCNConv + mean-pool + 2-layer MLP) on 8 Trainium2 cores.

Sharding strategy (graph/data parallel per the hint):
- Nodes partitioned contiguously: core c owns dst nodes [c*6250, (c+1)*6250).
- conv1 (aggregate-then-transform): edges + self-loops partitioned by dst
  owner, grouped into 98 windows of 64 dst nodes, padded to 128-edge chunks
  (chunk counts maxed across cores -> one SPMD program). The host ships each
  core its incident edges' x rows pre-scaled by the full sym-norm
  dinv[src]*dinv[dst] and quantized to fp8-e4m3 (chunk-ordered -> one big
  sequential DMA stream per batch of <=4 windows). The scatter-add is
  realized on the PE as matmuls with the fp8 x chunk stationary (FWL) and a
  64-wide 0/1 one-hot (iota-compare on DVE/Pool) as the moving operand,
  accumulating in PSUM -> the aggregation lands feature-major, no
  transposes. Dense W1 applied with fp8 DoubleRow matmuls (both 128-row
  k-tiles in one pass, W1 stationary) + bias + relu -> h1 kept feature-major
  in SBUF only (bf16).
- conv2 + mean-pool fused algebraically: with no nonlinearity between
  conv2's aggregation and the pooling, pooled sums satisfy
  pool[G] = sum_s A[s,G] * (h1[s] @ W2), where
  A[s,G] = dinv[s]*(sum_{e:src=s,dst in G} dinv[dst] + [batch[s]==G]*dinv[s])
  is built on host from edge_index/batch/deg only (structural data). Each
  core computes p = h1 @ W2 (bf16) for its own node chunks and immediately
  accumulates pb^T @ A_chunk into persistent [128,64] PSUM tiles, keeping
  the pooled partials FEATURE-major -- the tail MLP then needs no
  transposes at all.
- One 16KB fp8 AllReduce of the pooled partials at the end (two ARs
  serialize on the CC stream, so a single late one exposes less latency).
  mean+bias+relu and the tiny MLP run replicated in feature-major layout
  (out lands as [DOUT, NG] directly); core 0's output wins.
- Pipelining: per-batch x_edges DMA + one-hot build (triple buffered,
  issue-ahead 2), aggregation/dense/p-chunks interleaved batch by batch so
  the PE stays busy end to end.
"""

import sys
import types

import ml_dtypes
import numpy as np

try:
    import antenv  # noqa: F401

    if "antenv.axon_hooks" not in sys.modules:
        _m = types.ModuleType("antenv.axon_hooks")
        _m._hook = None
        _m.set_axon_ntff_profile_hook = lambda h: setattr(_m, "_hook", h)
        _m.get_axon_ntff_profile_hook = lambda: _m._hook
        sys.modules["antenv.axon_hooks"] = _m
except Exception:
    pass

import concourse.bacc as bacc
import concourse.mybir as mybir
import concourse.tile as tile
from concourse import bass_utils

F32 = mybir.dt.float32
BF16 = mybir.dt.bfloat16
F8 = mybir.dt.float8e4
AF = mybir.ActivationFunctionType
OP = mybir.AluOpType
DR = mybir.MatmulPerfMode.DoubleRow

N = 50000
E = 500000
DIN = 256
DH = 512
NG = 64
DOUT = 16

NCORES = 8
SLICE = N // NCORES  # 6250
WW = 64  # dst window width (one-hot width)
NW = (SLICE + WW - 1) // WW  # 98 windows
NPAD = 6272  # 49 * 128 node columns
NCHK = NPAD // 128  # 49 node chunks
NGRP = 13  # 12 groups of 512 node cols + 1 of 128

# tuning knobs
USE_DR_DENSE = True  # fp8 DoubleRow for the W1 dense
OH_ON_GPSIMD = False  # Pool engine lacks is_equal (walrus ISA check)
N_AR = 1  # number of pool AllReduce pieces (1 or 2)
AR_SPLIT = 24  # first-AR chunk split when N_AR == 2

_COMPILED: dict = {}


def _group_info(g):
    """(first window, #windows, node col0, #node cols, first chunk, #chunks)"""
    if g < 12:
        return (8 * g, 8, 512 * g, 512, 4 * g, 4)
    return (96, 2, 6144, 128, 48, 1)


def _layout(K1):
    """Batches of <=4 windows: [(g, ws, {w: [(gcol, grel)]}, nch, c0)]."""
    batches = []
    gcol = 0
    for g in range(NGRP):
        w0, nwin, _, _, _, _ = _group_info(g)
        nhalf = 2 if nwin == 8 else 1
        for half in range(nhalf):
            ws = list(range(w0 + half * 4, min(w0 + (half + 1) * 4, w0 + nwin)))
            c0 = gcol
            rel = 0
            wch = {}
            for w in ws:
                lst = []
                for _ in range(int(K1[w])):
                    lst.append((gcol, rel))
                    gcol += 1
                    rel += 1
                wch[w] = lst
            batches.append((g, ws, wch, rel, c0))
    return batches, gcol


def _preprocess(x, edge_index, batch):
    src = np.asarray(edge_index[0], dtype=np.int64)
    dst = np.asarray(edge_index[1], dtype=np.int64)
    batch = np.asarray(batch, dtype=np.int64)

    deg = np.bincount(dst, minlength=N).astype(np.float64) + 1.0
    dinv = (1.0 / np.sqrt(deg)).astype(np.float32)
    cnt = np.maximum(np.bincount(batch, minlength=NG), 1)

    loops = np.arange(N, dtype=np.int64)

    # ---------- conv1: edges + self-loops grouped by (core, 64-window) ----------
    s1 = np.concatenate([src, loops])
    d1 = np.concatenate([dst, loops])
    norm1 = dinv[s1] * dinv[d1]
    core1 = d1 // SLICE
    win1 = (d1 % SLICE) // WW
    key1 = core1 * NW + win1
    order1 = np.argsort(key1, kind="stable")
    ss1, ds1, nn1 = s1[order1], d1[order1], norm1[order1]
    counts1 = np.bincount(key1, minlength=NCORES * NW).reshape(NCORES, NW)
    starts1 = np.zeros(NCORES * NW + 1, dtype=np.int64)
    np.cumsum(counts1.reshape(-1), out=starts1[1:])
    K1 = np.ceil(counts1.max(axis=0) / 128).astype(np.int64)  # [NW]

    meta = tuple(int(v) for v in K1)
    batches, C1 = _layout(K1)

    # ---------- fused conv2+pool coefficient matrix A[s, G] ----------
    gd = batch[dst]
    A = np.bincount(src * NG + gd, weights=dinv[dst].astype(np.float64),
                    minlength=N * NG).reshape(N, NG).astype(np.float32)
    A[loops, batch] += dinv
    A *= dinv[:, None]
    # bake the mean-pool 1/cnt into A, x64 to keep fp8 AR payloads in range;
    # the tail activation divides by 64 via its scale parameter
    A *= (64.0 / np.maximum(cnt, 1).astype(np.float32))[None, :]

    xf = np.asarray(x, np.float32)

    per_core = []
    for c in range(NCORES):
        src_cols = np.zeros((C1, 128), dtype=np.int64)
        nrm_cols = np.zeros((C1, 128), dtype=np.float32)
        dst_cols = np.full((C1, 128), -1.0, dtype=np.float32)
        for _g, ws, wch, _nch, _c0 in batches:
            for w in ws:
                gi = c * NW + w
                e0, e1 = starts1[gi], starts1[gi + 1]
                n_e = int(e1 - e0)
                cols = wch[w]
                k = len(cols)
                sv = np.zeros(k * 128, dtype=np.int64)
                sv[:n_e] = ss1[e0:e1]
                nv = np.zeros(k * 128, dtype=np.float32)
                nv[:n_e] = nn1[e0:e1]
                dv = np.full(k * 128, -1.0, dtype=np.float32)
                dv[:n_e] = (ds1[e0:e1] - (c * SLICE + w * WW)).astype(np.float32)
                for j, (gcol, _r) in enumerate(cols):
                    src_cols[gcol] = sv[j * 128 : (j + 1) * 128]
                    nrm_cols[gcol] = nv[j * 128 : (j + 1) * 128]
                    dst_cols[gcol] = dv[j * 128 : (j + 1) * 128]
        rows = xf[src_cols.reshape(-1)] * nrm_cols.reshape(-1)[:, None]
        x_edges = np.ascontiguousarray(
            rows.astype(ml_dtypes.float8_e4m3).reshape(C1, 128, DIN).transpose(1, 0, 2)
        ).reshape(128, C1 * DIN)

        Ac = np.zeros((NPAD, NG), dtype=np.float32)
        Ac[:SLICE] = A[c * SLICE : (c + 1) * SLICE]
        a_sb = np.ascontiguousarray(
            Ac.reshape(NCHK, 128, NG).transpose(1, 0, 2)
        ).reshape(128, NCHK * NG).astype(ml_dtypes.bfloat16)

        per_core.append(
            dict(
                x_edges=x_edges,
                dst1=np.ascontiguousarray(dst_cols.T).astype(ml_dtypes.bfloat16),
                a_mat=a_sb,
            )
        )
    return meta, per_core, cnt.astype(np.float32)


def _build_program(meta):
    K1 = np.array(meta)
    batches, C1 = _layout(K1)
    max_nch = max(b[3] for b in batches)

    nc = bacc.Bacc("TRN2", target_bir_lowering=False, debug=False, num_devices=NCORES)

    def din(name, shape, dt=F32):
        return nc.dram_tensor(name, shape, dt, kind="ExternalInput").ap()

    x_edges = din("x_edges", [128, C1 * DIN], F8)
    dst1 = din("dst1", [128, C1], BF16)
    a_mat = din("a_mat", [128, NCHK * NG], BF16)
    iota_rep = din("iota_rep", [128, max_nch * WW], BF16)
    w1dr = din("w1dr", [128, 2 * DH], F8)  # [p, k(2), m(4), 128] fp8 pairs
    w1bf = din("w1bf", [128, 2 * DH], BF16)  # [p, k(2), fo(512)] bf16 fallback
    w2b = din("w2b", [128, 4 * (DH // 2)], BF16)
    b1c = din("b1c", [128, DH // 128])
    b2h = din("b2h", [128, 2])  # b2 feature-major halves
    wf1 = din("wf1", [128, 2 * (DH // 4)], BF16)  # [p, k(2), 128] bf16
    bf1c = din("bf1c", [128, 1])
    wf2 = din("wf2", [DH // 4, DOUT], BF16)
    bf2c = din("bf2c", [DOUT, 1])
    out = nc.dram_tensor("out", [DOUT, NG], F32, kind="ExternalOutput").ap()

    with tile.TileContext(nc) as tc:
        with (
            tc.tile_pool(name="const", bufs=1) as cp,
            tc.tile_pool(name="big", bufs=1) as bigp,
            tc.tile_pool(name="work", bufs=1) as wp,
            tc.tile_pool(name="psum", bufs=1, space="PSUM") as pp,
            tc.tile_pool(name="dram", bufs=1, space="DRAM") as dp,
        ):
            def load(ap_in, shape, dt=F32, pool=cp):
                t = pool.tile(shape, dt, name=ap_in.tensor.name + "_sb")
                nc.sync.dma_start(t[:], ap_in[:])
                return t

            # loads gating the pipeline start go first; the rest after batch 0
            dst1_sb = load(dst1, [128, C1], BF16)
            iota_sb = load(iota_rep, [128, max_nch * WW], BF16)

            h1s = [bigp.tile([128, NPAD], BF16, name=f"h1s_{k}") for k in range(4)]

            sfg_groups: dict = {}

            def sfg_of(g):
                # fp8 feature-major conv1 aggregation for group g: [p, k(2), n]
                if g not in sfg_groups:
                    sfg_groups[g] = wp.tile(
                        [128, 2, 512], F8, tag="sfg", bufs=2, name=f"sfg_{g}"
                    )
                return sfg_groups[g]

            # persistent feature-major pool partials: pgo[i][h] = [128 o, 64 G]
            pgo = [
                [
                    pp.tile([128, NG], F32, name=f"pgo_{i}_{h}")
                    for h in range(2)
                ]
                for i in range(N_AR)
            ]
            g_local = [dp.tile([128, 2 * NG], F8, name=f"gl{i}") for i in range(N_AR)]
            g_red = [
                dp.tile([128, 2 * NG], F8, addr_space="Shared", name=f"gr{i}")
                for i in range(N_AR)
            ]
            gs2 = [wp.tile([128, 2 * NG], F8, name=f"gs{i}") for i in range(N_AR)]

            def emit_allreduce(i):
                gsb = wp.tile([128, 2, NG], F8, name=f"gsb{i}")
                for h in range(2):
                    nc.vector.tensor_copy(gsb[:, h, :], pgo[i][h][:])
                nc.sync.dma_start(g_local[i][:], gsb[:].rearrange("p h g -> p (h g)"))
                nc.gpsimd.collective_compute(
                    "AllReduce",
                    OP.add,
                    replica_groups=[list(range(NCORES))],
                    ins=[g_local[i].opt()],
                    outs=[g_red[i].opt()],
                )
                nc.sync.dma_start(gs2[i][:], g_red[i][:])

            def emit_stream(bi):
                """One G1 DMA + one-hot build covering one batch (<=4 windows)."""
                _g, _ws, _wch, nch, c0 = batches[bi]
                G1 = wp.tile([128, nch, DIN], F8, tag="G1", bufs=3, name=f"g1b_{bi}")
                nc.sync.dma_start(
                    G1[:].rearrange("p c d -> p (c d)"),
                    x_edges[:, c0 * DIN : (c0 + nch) * DIN],
                )
                oh = wp.tile([128, nch, WW], BF16, tag="oh", bufs=3, name=f"ohb_{bi}")
                eng = nc.gpsimd if (OH_ON_GPSIMD and bi % 2 == 1) else nc.vector
                eng.tensor_tensor(
                    out=oh[:],
                    in0=iota_sb[:, : nch * WW].rearrange("p (c o) -> p c o", o=WW),
                    in1=dst1_sb[:, c0 : c0 + nch]
                    .rearrange("p (c o) -> p c o", o=1)
                    .to_broadcast([128, nch, WW]),
                    op=OP.is_equal,
                )
                return G1, oh

            def emit_batch(g, ws, wch, nch, c0, G1, oh):
                sfg = sfg_of(g)
                nw = len(ws)
                wb0 = ws[0] - 8 * g
                pa = pp.tile([128, nw, 2, WW], F32, tag="agg", bufs=2, name=f"pa_{ws[0]}")
                for w in ws:
                    cols = wch[w]
                    wrel = w - ws[0]
                    for j, (_gcol, grel) in enumerate(cols):
                        for h in range(2):
                            nc.tensor.matmul(
                                out=pa[:, wrel, h, :],
                                lhsT=G1[:, grel, h * 128 : (h + 1) * 128],
                                rhs=oh[:, grel, :],
                                start=(j == 0),
                                stop=(j == len(cols) - 1),
                            )
                for h in range(2):
                    nc.vector.tensor_copy(
                        sfg[:, h, wb0 * WW : (wb0 + nw) * WW],
                        pa[:, :, h, :],
                    )

            def emit_dense(g):
                _, _, n0, ncols, _, _ = _group_info(g)
                sfg = sfg_of(g)
                for m in range(4):
                    ph = pp.tile([128, 512], F32, tag="h1", bufs=2, name=f"ph_{g}_{m}")
                    if USE_DR_DENSE:
                        nc.tensor.matmul(
                            out=ph[:, :ncols],
                            lhsT=w1_sb[:, :, m, :],
                            rhs=sfg[:, :, :ncols],
                            start=True,
                            stop=True,
                            perf_mode=DR,
                        )
                    else:
                        for k in range(2):
                            nc.tensor.matmul(
                                out=ph[:, :ncols],
                                lhsT=w1f_sb[:, k, m * 128 : (m + 1) * 128],
                                rhs=sfg[:, k, :ncols],
                                start=(k == 0),
                                stop=(k == 1),
                            )
                    nc.scalar.activation(
                        h1s[m][:, n0 : n0 + ncols], ph[:, :ncols], AF.Relu,
                        bias=b1_sb[:, m : m + 1],
                    )

            def emit_pA(cc):
                c0 = cc * 128
                ppm = pp.tile([128, DH // 2], F32, tag="p2", bufs=2, name=f"ppm_{cc}")
                for k in range(4):
                    nc.tensor.matmul(
                        out=ppm[:],
                        lhsT=h1s[k][:, c0 : c0 + 128],
                        rhs=w2_sb[:, k * (DH // 2) : (k + 1) * (DH // 2)],
                        start=(k == 0),
                        stop=(k == 3),
                    )
                pb = wp.tile([128, DH // 2], BF16, tag="pb", bufs=2, name=f"pb_{cc}")
                nc.scalar.activation(pb[:], ppm[:], AF.Copy)
                ar = 0 if (N_AR == 1 or cc < AR_SPLIT) else 1
                first = cc == 0 or (ar == 1 and cc == AR_SPLIT)
                last = cc == NCHK - 1 or (ar == 0 and N_AR == 2 and cc == AR_SPLIT - 1)
                for h in range(2):
                    nc.tensor.matmul(
                        out=pgo[ar][h][:],
                        lhsT=pb[:, h * 128 : (h + 1) * 128],
                        rhs=a_sb[:, cc * NG : (cc + 1) * NG],
                        start=first,
                        stop=last,
                    )

            streams = {0: emit_stream(0), 1: emit_stream(1)}
            if USE_DR_DENSE:
                w1_sb = load(w1dr, [128, 2, 4, 128], F8)
            else:
                w1f_sb = load(w1bf, [128, 2, DH], BF16)
            b1_sb = load(b1c, [128, DH // 128])
            a_sb = load(a_mat, [128, NCHK * NG], BF16)
            w2_sb = load(w2b, [128, 4 * (DH // 2)], BF16)
            b2_sb = load(b2h, [128, 2])
            wf1_sb = load(wf1, [128, 2, DH // 4], BF16)
            bf1_sb = load(bf1c, [128, 1])
            wf2_sb = load(wf2, [DH // 4, DOUT], BF16)
            bf2_sb = load(bf2c, [DOUT, 1])
            # warm-up collective: absorbs the CC barrier + cold firmware setup
            # during the main phase so the real AllReduce launches warm
            warm_l = dp.tile([128, 8], F8, name="warm_l")
            warm_r = dp.tile([128, 8], F8, addr_space="Shared", name="warm_r")
            nc.gpsimd.collective_compute(
                "AllReduce",
                OP.add,
                replica_groups=[list(range(NCORES))],
                ins=[warm_l.opt()],
                outs=[warm_r.opt()],
            )
            bidx = 0
            for g in range(NGRP):
                _, nwin, _, _, cc0, nccs = _group_info(g)
                nb = 2 if nwin == 8 else 1
                for _b in range(nb):
                    if bidx + 2 < len(batches):
                        streams[bidx + 2] = emit_stream(bidx + 2)
                    G1, oh = streams.pop(bidx)
                    emit_batch(*batches[bidx], G1, oh)
                    bidx += 1
                emit_dense(g)
                for cc in range(cc0, cc0 + nccs):
                    emit_pA(cc)
                    if N_AR == 2 and cc == AR_SPLIT - 1:
                        emit_allreduce(0)

            # -------- tail: AllReduce + mean/bias/relu + MLP, all feature-major
            # (the 1/cnt mean is baked into A on host, x64; undone via scale)
            emit_allreduce(N_AR - 1)
            if N_AR == 2:
                cur = wp.tile([128, 2 * NG], F32, name="gsum")
                nc.vector.tensor_tensor(
                    out=cur[:], in0=gs2[0][:], in1=gs2[1][:], op=OP.add
                )
                curv = cur[:].rearrange("p (h g) -> p h g", h=2)
            else:
                curv = gs2[0][:].rearrange("p (h g) -> p h g", h=2)
            grelu = wp.tile([128, 2, NG], BF16, name="grelu")
            for h in range(2):
                nc.scalar.activation(
                    grelu[:, h, :], curv[:, h, :], AF.Relu,
                    bias=b2_sb[:, h : h + 1], scale=1.0 / 64.0,
                )
            pz = pp.tile([128, NG], F32, tag="p2", bufs=2, name="pz")
            for k in range(2):
                nc.tensor.matmul(
                    out=pz[:],
                    lhsT=wf1_sb[:, k, :],
                    rhs=grelu[:, k, :],
                    start=(k == 0),
                    stop=(k == 1),
                )
            zsb = wp.tile([128, NG], BF16, name="zsb")
            nc.scalar.activation(zsb[:], pz[:], AF.Relu, bias=bf1_sb[:, 0:1])
            po = pp.tile([DOUT, NG], F32, tag="agg", bufs=2, name="po")
            nc.tensor.matmul(out=po[:], lhsT=wf2_sb[:], rhs=zsb[:], start=True, stop=True)
            osb = wp.tile([DOUT, NG], F32, name="osb")
            nc.scalar.activation(osb[:], po[:], AF.Relu, bias=bf2_sb[:, 0:1])
            nc.sync.dma_start(out[:], osb[:])

    nc.compile()
    return nc


def _get_program(meta):
    if meta not in _COMPILED:
        _COMPILED[meta] = _build_program(meta)
    return _COMPILED[meta]


def _make_in_maps(W1, b1, W2, b2, Wf1, bf1, Wf2, bf2, per_core, cnt, meta):
    bf = ml_dtypes.bfloat16
    f8 = ml_dtypes.float8_e4m3
    W1 = np.asarray(W1, np.float32)
    W2 = np.asarray(W2, np.float32)
    Wf1 = np.asarray(Wf1, np.float32)
    b2 = np.asarray(b2, np.float32)
    K1 = np.array(meta)
    batches, _C1 = _layout(K1)
    max_nch = max(b[3] for b in batches)

    # w1dr[p, k, m, c] = W1[k*128+p, m*128+c]
    w1dr = np.ascontiguousarray(
        W1.reshape(2, 128, 4, 128).transpose(1, 0, 2, 3).reshape(128, 2 * DH)
    )
    w1bf = np.ascontiguousarray(
        W1.reshape(2, 128, DH).transpose(1, 0, 2).reshape(128, 2 * DH)
    )
    shared = dict(
        iota_rep=np.ascontiguousarray(
            np.tile(np.arange(WW, dtype=np.float32)[None, None, :],
                    (128, max_nch, 1)).reshape(128, max_nch * WW)
        ).astype(bf),
        w1dr=w1dr.astype(f8),
        w1bf=w1bf.astype(bf),
        w2b=np.ascontiguousarray(
            np.concatenate([W2[k * 128 : (k + 1) * 128, :] for k in range(4)], axis=1)
        ).astype(bf),
        b1c=np.ascontiguousarray(np.asarray(b1, np.float32).reshape(DH // 128, 128).T),
        b2h=np.ascontiguousarray(b2.reshape(2, 128).T),
        wf1=np.ascontiguousarray(
            Wf1.reshape(2, 128, DH // 4).transpose(1, 0, 2).reshape(128, 2 * (DH // 4))
        ).astype(bf),
        bf1c=np.tile(np.asarray(bf1, np.float32).reshape(DH // 4, 1), (1, 1)),
        wf2=np.asarray(Wf2, np.float32).astype(bf),
        bf2c=np.asarray(bf2, np.float32).reshape(DOUT, 1),
    )
    return [dict(shared, **per_core[c]) for c in range(NCORES)]


def kernel(
    x, W1, b1, W2, b2, Wf1, bf1, Wf2, bf2, edge_index, batch, num_graphs, _trace=False
):
    assert int(num_graphs) == NG
    meta, per_core, cnt = _preprocess(
        np.asarray(x), np.asarray(edge_index), np.asarray(batch)
    )
    nc = _get_program(meta)
    in_maps = _make_in_maps(W1, b1, W2, b2, Wf1, bf1, Wf2, bf2, per_core, cnt, meta)
    res = bass_utils.run_bass_kernel_spmd(
        nc, in_maps, core_ids=list(range(NCORES)), trace=_trace
    )
    out = np.ascontiguousarray(np.asarray(res.results[0]["out"], np.float32).T)
    if _trace:
        kernel._last_results = res
    return out


# revision 25
# speedup vs baseline: 1.1501x; 1.0188x over previous
"""GCN classifier (2x GCNConv + mean-pool + 2-layer MLP) on 8 Trainium2 cores.

Sharding strategy (graph/data parallel per the hint):
- Nodes partitioned contiguously: core c owns dst nodes [c*6250, (c+1)*6250).
- conv1 (aggregate-then-transform): edges + self-loops partitioned by dst
  owner, grouped into 98 windows of 64 dst nodes, padded to 128-edge chunks
  (chunk counts maxed across cores -> one SPMD program). The host ships each
  core its incident edges' x rows pre-scaled by the full sym-norm
  dinv[src]*dinv[dst] and quantized to fp8-e4m3 (chunk-ordered -> one big
  sequential DMA stream per batch of <=4 windows). The scatter-add is
  realized on the PE as matmuls with the fp8 x chunk stationary (FWL) and a
  64-wide 0/1 one-hot (iota-compare on DVE/Pool) as the moving operand,
  accumulating in PSUM -> the aggregation lands feature-major, no
  transposes. Dense W1 applied with fp8 DoubleRow matmuls (both 128-row
  k-tiles in one pass, W1 stationary) + bias + relu -> h1 kept feature-major
  in SBUF only (bf16).
- conv2 + mean-pool fused algebraically: with no nonlinearity between
  conv2's aggregation and the pooling, pooled sums satisfy
  pool[G] = sum_s A[s,G] * (h1[s] @ W2), where
  A[s,G] = dinv[s]*(sum_{e:src=s,dst in G} dinv[dst] + [batch[s]==G]*dinv[s])
  is built on host from edge_index/batch/deg only (structural data). Each
  core computes p = h1 @ W2 (bf16) for its own node chunks and immediately
  accumulates pb^T @ A_chunk into persistent [128,64] PSUM tiles, keeping
  the pooled partials FEATURE-major -- the tail MLP then needs no
  transposes at all.
- One 16KB fp8 AllReduce of the pooled partials at the end (two ARs
  serialize on the CC stream, so a single late one exposes less latency).
  mean+bias+relu and the tiny MLP run replicated in feature-major layout
  (out lands as [DOUT, NG] directly); core 0's output wins.
- Pipelining: per-batch x_edges DMA + one-hot build (triple buffered,
  issue-ahead 2), aggregation/dense/p-chunks interleaved batch by batch so
  the PE stays busy end to end.
"""

import sys
import types

import ml_dtypes
import numpy as np

try:
    import antenv  # noqa: F401

    if "antenv.axon_hooks" not in sys.modules:
        _m = types.ModuleType("antenv.axon_hooks")
        _m._hook = None
        _m.set_axon_ntff_profile_hook = lambda h: setattr(_m, "_hook", h)
        _m.get_axon_ntff_profile_hook = lambda: _m._hook
        sys.modules["antenv.axon_hooks"] = _m
except Exception:
    pass

import concourse.bacc as bacc
import concourse.mybir as mybir
import concourse.tile as tile
from concourse import bass_utils

F32 = mybir.dt.float32
BF16 = mybir.dt.bfloat16
F8 = mybir.dt.float8e4
AF = mybir.ActivationFunctionType
OP = mybir.AluOpType
DR = mybir.MatmulPerfMode.DoubleRow

N = 50000
E = 500000
DIN = 256
DH = 512
NG = 64
DOUT = 16

NCORES = 8
SLICE = N // NCORES  # 6250
WW = 64  # dst window width (one-hot width)
NW = (SLICE + WW - 1) // WW  # 98 windows
NPAD = 6272  # 49 * 128 node columns
NCHK = NPAD // 128  # 49 node chunks
NGRP = 13  # 12 groups of 512 node cols + 1 of 128

# tuning knobs
USE_DR_DENSE = True  # fp8 DoubleRow for the W1 dense
OH_ON_GPSIMD = False  # Pool engine lacks is_equal (walrus ISA check)
N_AR = 1  # number of pool AllReduce pieces (1 or 2)
AR_SPLIT = 24  # first-AR chunk split when N_AR == 2

_COMPILED: dict = {}


def _group_info(g):
    """(first window, #windows, node col0, #node cols, first chunk, #chunks)"""
    if g < 12:
        return (8 * g, 8, 512 * g, 512, 4 * g, 4)
    return (96, 2, 6144, 128, 48, 1)


def _layout(K1):
    """Batches of <=4 windows: [(g, ws, {w: [(gcol, grel)]}, nch, c0)]."""
    batches = []
    gcol = 0
    for g in range(NGRP):
        w0, nwin, _, _, _, _ = _group_info(g)
        nhalf = 2 if nwin == 8 else 1
        for half in range(nhalf):
            ws = list(range(w0 + half * 4, min(w0 + (half + 1) * 4, w0 + nwin)))
            c0 = gcol
            rel = 0
            wch = {}
            for w in ws:
                lst = []
                for _ in range(int(K1[w])):
                    lst.append((gcol, rel))
                    gcol += 1
                    rel += 1
                wch[w] = lst
            batches.append((g, ws, wch, rel, c0))
    return batches, gcol


def _preprocess(x, edge_index, batch):
    src = np.asarray(edge_index[0], dtype=np.int64)
    dst = np.asarray(edge_index[1], dtype=np.int64)
    batch = np.asarray(batch, dtype=np.int64)

    deg = np.bincount(dst, minlength=N).astype(np.float64) + 1.0
    dinv = (1.0 / np.sqrt(deg)).astype(np.float32)
    cnt = np.maximum(np.bincount(batch, minlength=NG), 1)

    loops = np.arange(N, dtype=np.int64)

    # ---------- conv1: edges + self-loops grouped by (core, 64-window) ----------
    s1 = np.concatenate([src, loops])
    d1 = np.concatenate([dst, loops])
    norm1 = dinv[s1] * dinv[d1]
    core1 = d1 // SLICE
    win1 = (d1 % SLICE) // WW
    key1 = core1 * NW + win1
    order1 = np.argsort(key1, kind="stable")
    ss1, ds1, nn1 = s1[order1], d1[order1], norm1[order1]
    counts1 = np.bincount(key1, minlength=NCORES * NW).reshape(NCORES, NW)
    starts1 = np.zeros(NCORES * NW + 1, dtype=np.int64)
    np.cumsum(counts1.reshape(-1), out=starts1[1:])
    K1 = np.ceil(counts1.max(axis=0) / 128).astype(np.int64)  # [NW]

    meta = tuple(int(v) for v in K1)
    batches, C1 = _layout(K1)

    # ---------- fused conv2+pool coefficient matrix A[s, G] ----------
    gd = batch[dst]
    A = np.bincount(src * NG + gd, weights=dinv[dst].astype(np.float64),
                    minlength=N * NG).reshape(N, NG).astype(np.float32)
    A[loops, batch] += dinv
    A *= dinv[:, None]
    # bake the mean-pool 1/cnt into A, x64 to keep fp8 AR payloads in range;
    # the tail activation divides by 64 via its scale parameter
    A *= (64.0 / np.maximum(cnt, 1).astype(np.float32))[None, :]

    xf = np.asarray(x, np.float32)

    per_core = []
    for c in range(NCORES):
        src_cols = np.zeros((C1, 128), dtype=np.int64)
        nrm_cols = np.zeros((C1, 128), dtype=np.float32)
        dst_cols = np.full((C1, 128), -1.0, dtype=np.float32)
        for _g, ws, wch, _nch, _c0 in batches:
            for w in ws:
                gi = c * NW + w
                e0, e1 = starts1[gi], starts1[gi + 1]
                n_e = int(e1 - e0)
                cols = wch[w]
                k = len(cols)
                sv = np.zeros(k * 128, dtype=np.int64)
                sv[:n_e] = ss1[e0:e1]
                nv = np.zeros(k * 128, dtype=np.float32)
                nv[:n_e] = nn1[e0:e1]
                dv = np.full(k * 128, -1.0, dtype=np.float32)
                dv[:n_e] = (ds1[e0:e1] - (c * SLICE + w * WW)).astype(np.float32)
                for j, (gcol, _r) in enumerate(cols):
                    src_cols[gcol] = sv[j * 128 : (j + 1) * 128]
                    nrm_cols[gcol] = nv[j * 128 : (j + 1) * 128]
                    dst_cols[gcol] = dv[j * 128 : (j + 1) * 128]
        rows = xf[src_cols.reshape(-1)] * nrm_cols.reshape(-1)[:, None]
        x_edges = np.ascontiguousarray(
            rows.astype(ml_dtypes.float8_e4m3).reshape(C1, 128, DIN).transpose(1, 0, 2)
        ).reshape(128, C1 * DIN)

        Ac = np.zeros((NPAD, NG), dtype=np.float32)
        Ac[:SLICE] = A[c * SLICE : (c + 1) * SLICE]
        a_sb = np.ascontiguousarray(
            Ac.reshape(NCHK, 128, NG).transpose(1, 0, 2)
        ).reshape(128, NCHK * NG).astype(ml_dtypes.bfloat16)

        per_core.append(
            dict(
                x_edges=x_edges,
                dst1=np.ascontiguousarray(dst_cols.T).astype(ml_dtypes.bfloat16),
                a_mat=a_sb,
            )
        )
    return meta, per_core, cnt.astype(np.float32)


def _build_program(meta):
    K1 = np.array(meta)
    batches, C1 = _layout(K1)
    max_nch = max(b[3] for b in batches)

    nc = bacc.Bacc("TRN2", target_bir_lowering=False, debug=False, num_devices=NCORES)

    def din(name, shape, dt=F32):
        return nc.dram_tensor(name, shape, dt, kind="ExternalInput").ap()

    x_edges = din("x_edges", [128, C1 * DIN], F8)
    dst1 = din("dst1", [128, C1], BF16)
    a_mat = din("a_mat", [128, NCHK * NG], BF16)
    iota64 = din("iota64", [128, WW], BF16)
    w1dr = din("w1dr", [128, 2 * DH], F8)  # [p, k(2), m(4), 128] fp8 pairs
    w1bf = din("w1bf", [128, 2 * DH], BF16)  # [p, k(2), fo(512)] bf16 fallback
    # merged bf16 consts: [0:1024] w2b, [1024:1280] wf1, [1280:1296] wf2
    wc_bf = din("wc_bf", [128, 1296], BF16)
    # merged f32 consts: [0:4] b1c, [4:6] b2h, [6:7] bf1c, [7:8] bf2c (rows<16)
    fc32 = din("fc32", [128, 8])
    out = nc.dram_tensor("out", [DOUT, NG], F32, kind="ExternalOutput").ap()

    with tile.TileContext(nc) as tc:
        with (
            tc.tile_pool(name="const", bufs=1) as cp,
            tc.tile_pool(name="big", bufs=1) as bigp,
            tc.tile_pool(name="work", bufs=1) as wp,
            tc.tile_pool(name="psum", bufs=1, space="PSUM") as pp,
            tc.tile_pool(name="dram", bufs=1, space="DRAM") as dp,
        ):
            def load(ap_in, shape, dt=F32, pool=cp):
                t = pool.tile(shape, dt, name=ap_in.tensor.name + "_sb")
                nc.sync.dma_start(t[:], ap_in[:])
                return t

            # loads gating the pipeline start go first; the rest after batch 0
            dst1_sb = load(dst1, [128, C1], BF16)
            iota_sb = load(iota64, [128, WW], BF16)

            h1s = [bigp.tile([128, NPAD], BF16, name=f"h1s_{k}") for k in range(4)]

            sfg_groups: dict = {}

            def sfg_of(g):
                # fp8 feature-major conv1 aggregation for group g: [p, k(2), n]
                if g not in sfg_groups:
                    sfg_groups[g] = wp.tile(
                        [128, 2, 512], F8, tag="sfg", bufs=2, name=f"sfg_{g}"
                    )
                return sfg_groups[g]

            # persistent feature-major pool partials: pgo[i][h] = [128 o, 64 G]
            pgo = [
                [
                    pp.tile([128, NG], F32, name=f"pgo_{i}_{h}")
                    for h in range(2)
                ]
                for i in range(N_AR)
            ]
            g_local = [dp.tile([128, 2 * NG], F8, name=f"gl{i}") for i in range(N_AR)]
            g_red = [
                dp.tile([128, 2 * NG], F8, addr_space="Shared", name=f"gr{i}")
                for i in range(N_AR)
            ]
            gs2 = [wp.tile([128, 2 * NG], F8, name=f"gs{i}") for i in range(N_AR)]

            def emit_allreduce(i):
                gsb = wp.tile([128, 2, NG], F8, name=f"gsb{i}")
                for h in range(2):
                    nc.vector.tensor_copy(gsb[:, h, :], pgo[i][h][:])
                nc.sync.dma_start(g_local[i][:], gsb[:].rearrange("p h g -> p (h g)"))
                nc.gpsimd.collective_compute(
                    "AllReduce",
                    OP.add,
                    replica_groups=[list(range(NCORES))],
                    ins=[g_local[i].opt()],
                    outs=[g_red[i].opt()],
                )
                nc.sync.dma_start(gs2[i][:], g_red[i][:])

            def emit_stream(bi):
                """One G1 DMA + one-hot build covering one batch (<=4 windows)."""
                _g, _ws, _wch, nch, c0 = batches[bi]
                G1 = wp.tile([128, nch, DIN], F8, tag="G1", bufs=3, name=f"g1b_{bi}")
                nc.sync.dma_start(
                    G1[:].rearrange("p c d -> p (c d)"),
                    x_edges[:, c0 * DIN : (c0 + nch) * DIN],
                )
                oh = wp.tile([128, nch, WW], BF16, tag="oh", bufs=3, name=f"ohb_{bi}")
                eng = nc.gpsimd if (OH_ON_GPSIMD and bi % 2 == 1) else nc.vector
                eng.tensor_tensor(
                    out=oh[:],
                    in0=iota_sb[:]
                    .rearrange("p (c o) -> p c o", c=1)
                    .to_broadcast([128, nch, WW]),
                    in1=dst1_sb[:, c0 : c0 + nch]
                    .rearrange("p (c o) -> p c o", o=1)
                    .to_broadcast([128, nch, WW]),
                    op=OP.is_equal,
                )
                return G1, oh

            def emit_batch(g, ws, wch, nch, c0, G1, oh):
                sfg = sfg_of(g)
                nw = len(ws)
                wb0 = ws[0] - 8 * g
                pa = pp.tile([128, nw, 2, WW], F32, tag="agg", bufs=2, name=f"pa_{ws[0]}")
                for w in ws:
                    cols = wch[w]
                    wrel = w - ws[0]
                    for j, (_gcol, grel) in enumerate(cols):
                        for h in range(2):
                            nc.tensor.matmul(
                                out=pa[:, wrel, h, :],
                                lhsT=G1[:, grel, h * 128 : (h + 1) * 128],
                                rhs=oh[:, grel, :],
                                start=(j == 0),
                                stop=(j == len(cols) - 1),
                            )
                for h in range(2):
                    nc.scalar.activation(
                        sfg[:, h, wb0 * WW : (wb0 + nw) * WW],
                        pa[:, :, h, :],
                        AF.Copy,
                    )

            def emit_dense(g):
                _, _, n0, ncols, _, _ = _group_info(g)
                sfg = sfg_of(g)
                for m in range(4):
                    ph = pp.tile([128, 512], F32, tag="h1", bufs=2, name=f"ph_{g}_{m}")
                    if USE_DR_DENSE:
                        nc.tensor.matmul(
                            out=ph[:, :ncols],
                            lhsT=w1_sb[:, :, m, :],
                            rhs=sfg[:, :, :ncols],
                            start=True,
                            stop=True,
                            perf_mode=DR,
                        )
                    else:
                        for k in range(2):
                            nc.tensor.matmul(
                                out=ph[:, :ncols],
                                lhsT=w1f_sb[:, k, m * 128 : (m + 1) * 128],
                                rhs=sfg[:, k, :ncols],
                                start=(k == 0),
                                stop=(k == 1),
                            )
                    nc.scalar.activation(
                        h1s[m][:, n0 : n0 + ncols], ph[:, :ncols], AF.Relu,
                        bias=fc_sb[:, m : m + 1],
                    )

            def emit_pA(cc):
                c0 = cc * 128
                ppm = pp.tile([128, DH // 2], F32, tag="p2", bufs=2, name=f"ppm_{cc}")
                for k in range(4):
                    nc.tensor.matmul(
                        out=ppm[:],
                        lhsT=h1s[k][:, c0 : c0 + 128],
                        rhs=wc_sb[:, k * (DH // 2) : (k + 1) * (DH // 2)],
                        start=(k == 0),
                        stop=(k == 3),
                    )
                pb = wp.tile([128, DH // 2], BF16, tag="pb", bufs=2, name=f"pb_{cc}")
                nc.vector.tensor_copy(pb[:], ppm[:])
                ar = 0 if (N_AR == 1 or cc < AR_SPLIT) else 1
                first = cc == 0 or (ar == 1 and cc == AR_SPLIT)
                last = cc == NCHK - 1 or (ar == 0 and N_AR == 2 and cc == AR_SPLIT - 1)
                for h in range(2):
                    nc.tensor.matmul(
                        out=pgo[ar][h][:],
                        lhsT=pb[:, h * 128 : (h + 1) * 128],
                        rhs=a_sb[:, cc * NG : (cc + 1) * NG],
                        start=first,
                        stop=last,
                    )

            streams = {0: emit_stream(0), 1: emit_stream(1)}
            if USE_DR_DENSE:
                w1_sb = load(w1dr, [128, 2, 4, 128], F8)
            else:
                w1f_sb = load(w1bf, [128, 2, DH], BF16)
            fc_sb = load(fc32, [128, 8])
            # warm-up collectives: absorb the CC barrier + cold firmware setup
            # during the main phase so the real AllReduce launches warm. The
            # extra probes measure RS/AG cost off the critical path.
            warm_l = dp.tile([128, 128], F8, name="warm_l")
            warm_r = dp.tile([128, 128], F8, addr_space="Shared", name="warm_r")
            warm_rs = dp.tile([16, 128], F8, name="warm_rs")
            warm_ag = dp.tile([128 * 8, 128], F8, addr_space="Shared", name="warm_ag")
            nc.gpsimd.collective_compute(
                "AllReduce", OP.add, replica_groups=[list(range(NCORES))],
                ins=[warm_l.opt()], outs=[warm_r.opt()],
            )
            nc.gpsimd.collective_compute(
                "ReduceScatter", OP.add, replica_groups=[list(range(NCORES))],
                ins=[warm_l.opt()], outs=[warm_rs.opt()],
            )
            nc.gpsimd.collective_compute(
                "AllGather", OP.bypass, replica_groups=[list(range(NCORES))],
                ins=[warm_l.opt()], outs=[warm_ag.opt()],
            )
            bidx = 0
            for g in range(NGRP):
                _, nwin, _, _, cc0, nccs = _group_info(g)
                nb = 2 if nwin == 8 else 1
                for _b in range(nb):
                    if bidx + 2 < len(batches):
                        streams[bidx + 2] = emit_stream(bidx + 2)
                    G1, oh = streams.pop(bidx)
                    emit_batch(*batches[bidx], G1, oh)
                    bidx += 1
                    if bidx == 1:
                        a_sb = load(a_mat, [128, NCHK * NG], BF16)
                        wc_sb = load(wc_bf, [128, 1296], BF16)
                emit_dense(g)
                for cc in range(cc0, cc0 + nccs):
                    emit_pA(cc)
                    if N_AR == 2 and cc == AR_SPLIT - 1:
                        emit_allreduce(0)

            # -------- tail: AllReduce + mean/bias/relu + MLP, all feature-major
            # (the 1/cnt mean is baked into A on host, x64; undone via scale)
            emit_allreduce(N_AR - 1)
            if N_AR == 2:
                cur = wp.tile([128, 2 * NG], F32, name="gsum")
                nc.vector.tensor_tensor(
                    out=cur[:], in0=gs2[0][:], in1=gs2[1][:], op=OP.add
                )
                curv = cur[:].rearrange("p (h g) -> p h g", h=2)
            else:
                curv = gs2[0][:].rearrange("p (h g) -> p h g", h=2)
            grelu = wp.tile([128, 2, NG], BF16, name="grelu")
            for h in range(2):
                nc.scalar.activation(
                    grelu[:, h, :], curv[:, h, :], AF.Relu,
                    bias=fc_sb[:, 4 + h : 5 + h], scale=1.0 / 64.0,
                )
            pz = pp.tile([128, NG], F32, tag="p2", bufs=2, name="pz")
            for k in range(2):
                nc.tensor.matmul(
                    out=pz[:],
                    lhsT=wc_sb[:, 1024 + k * 128 : 1024 + (k + 1) * 128],
                    rhs=grelu[:, k, :],
                    start=(k == 0),
                    stop=(k == 1),
                )
            zsb = wp.tile([128, NG], BF16, name="zsb")
            nc.scalar.activation(zsb[:], pz[:], AF.Relu, bias=fc_sb[:, 6:7])
            po = pp.tile([DOUT, NG], F32, tag="agg", bufs=2, name="po")
            nc.tensor.matmul(
                out=po[:], lhsT=wc_sb[:, 1280:1296], rhs=zsb[:], start=True, stop=True
            )
            osb = wp.tile([DOUT, NG], F32, name="osb")
            nc.scalar.activation(osb[:], po[:], AF.Relu, bias=fc_sb[:16, 7:8])
            nc.sync.dma_start(out[:], osb[:])

    nc.compile()
    return nc


def _get_program(meta):
    if meta not in _COMPILED:
        _COMPILED[meta] = _build_program(meta)
    return _COMPILED[meta]


def _make_in_maps(W1, b1, W2, b2, Wf1, bf1, Wf2, bf2, per_core, cnt, meta):
    bf = ml_dtypes.bfloat16
    f8 = ml_dtypes.float8_e4m3
    W1 = np.asarray(W1, np.float32)
    W2 = np.asarray(W2, np.float32)
    Wf1 = np.asarray(Wf1, np.float32)
    b2 = np.asarray(b2, np.float32)
    K1 = np.array(meta)
    batches, _C1 = _layout(K1)
    max_nch = max(b[3] for b in batches)

    # w1dr[p, k, m, c] = W1[k*128+p, m*128+c]
    w1dr = np.ascontiguousarray(
        W1.reshape(2, 128, 4, 128).transpose(1, 0, 2, 3).reshape(128, 2 * DH)
    )
    w1bf = np.ascontiguousarray(
        W1.reshape(2, 128, DH).transpose(1, 0, 2).reshape(128, 2 * DH)
    )
    w2b = np.ascontiguousarray(
        np.concatenate([W2[k * 128 : (k + 1) * 128, :] for k in range(4)], axis=1)
    )
    wf1b = np.ascontiguousarray(
        Wf1.reshape(2, 128, DH // 4).transpose(1, 0, 2).reshape(128, 2 * (DH // 4))
    )
    wc = np.concatenate([w2b, wf1b, np.asarray(Wf2, np.float32)], axis=1)
    fc = np.zeros((128, 8), np.float32)
    fc[:, 0:4] = np.asarray(b1, np.float32).reshape(DH // 128, 128).T
    fc[:, 4:6] = b2.reshape(2, 128).T
    fc[:, 6] = np.asarray(bf1, np.float32).reshape(DH // 4)
    fc[:DOUT, 7] = np.asarray(bf2, np.float32).reshape(DOUT)
    shared = dict(
        iota64=np.tile(np.arange(WW, dtype=np.float32)[None, :], (128, 1)).astype(bf),
        w1dr=w1dr.astype(f8),
        w1bf=w1bf.astype(bf),
        wc_bf=np.ascontiguousarray(wc).astype(bf),
        fc32=fc,
    )
    return [dict(shared, **per_core[c]) for c in range(NCORES)]


def kernel(
    x, W1, b1, W2, b2, Wf1, bf1, Wf2, bf2, edge_index, batch, num_graphs, _trace=False
):
    assert int(num_graphs) == NG
    meta, per_core, cnt = _preprocess(
        np.asarray(x), np.asarray(edge_index), np.asarray(batch)
    )
    nc = _get_program(meta)
    in_maps = _make_in_maps(W1, b1, W2, b2, Wf1, bf1, Wf2, bf2, per_core, cnt, meta)
    res = bass_utils.run_bass_kernel_spmd(
        nc, in_maps, core_ids=list(range(NCORES)), trace=_trace
    )
    out = np.ascontiguousarray(np.asarray(res.results[0]["out"], np.float32).T)
    if _trace:
        kernel._last_results = res
    return out


# revision 36
# speedup vs baseline: 1.2858x; 1.1180x over previous
"""GCN classifier (2x GCNConv + mean-pool + 2-layer MLP) on 8 Trainium2 cores.

Sharding strategy (graph/data parallel per the hint):
- Nodes partitioned contiguously: core c owns dst nodes [c*6250, (c+1)*6250).
- conv1 (aggregate-then-transform): edges + self-loops partitioned by dst
  owner, grouped into 98 windows of 64 dst nodes, padded to 128-edge chunks
  (chunk counts maxed across cores -> one SPMD program). The host ships each
  core its incident edges' x rows pre-scaled by the full sym-norm
  dinv[src]*dinv[dst] and quantized to fp8-e4m3 (chunk-ordered -> one big
  sequential DMA stream per batch of <=4 windows). The scatter-add is
  realized on the PE as matmuls with the fp8 x chunk stationary (FWL) and a
  64-wide 0/1 one-hot (iota-compare on DVE/Pool) as the moving operand,
  accumulating in PSUM -> the aggregation lands feature-major, no
  transposes. Dense W1 applied with fp8 DoubleRow matmuls (both 128-row
  k-tiles in one pass, W1 stationary) + bias + relu -> h1 kept feature-major
  in SBUF only (bf16).
- conv2 + mean-pool fused algebraically: with no nonlinearity between
  conv2's aggregation and the pooling, pooled sums satisfy
  pool[G] = sum_s A[s,G] * (h1[s] @ W2), where
  A[s,G] = dinv[s]*(sum_{e:src=s,dst in G} dinv[dst] + [batch[s]==G]*dinv[s])
  is built on host from edge_index/batch/deg only (structural data). Each
  core computes p = h1 @ W2 (bf16) for its own node chunks and immediately
  accumulates pb^T @ A_chunk into persistent [128,64] PSUM tiles, keeping
  the pooled partials FEATURE-major -- the tail MLP then needs no
  transposes at all.
- One 16KB fp8 AllReduce of the pooled partials at the end (two ARs
  serialize on the CC stream, so a single late one exposes less latency).
  mean+bias+relu and the tiny MLP run replicated in feature-major layout
  (out lands as [DOUT, NG] directly); core 0's output wins.
- Pipelining: per-batch x_edges DMA + one-hot build (triple buffered,
  issue-ahead 2), aggregation/dense/p-chunks interleaved batch by batch so
  the PE stays busy end to end.
"""

import sys
import types

import ml_dtypes
import numpy as np

try:
    import antenv  # noqa: F401

    if "antenv.axon_hooks" not in sys.modules:
        _m = types.ModuleType("antenv.axon_hooks")
        _m._hook = None
        _m.set_axon_ntff_profile_hook = lambda h: setattr(_m, "_hook", h)
        _m.get_axon_ntff_profile_hook = lambda: _m._hook
        sys.modules["antenv.axon_hooks"] = _m
except Exception:
    pass

import concourse.bacc as bacc
import concourse.mybir as mybir
import concourse.tile as tile
from concourse import bass_utils

F32 = mybir.dt.float32
BF16 = mybir.dt.bfloat16
F8 = mybir.dt.float8e4
AF = mybir.ActivationFunctionType
OP = mybir.AluOpType
DR = mybir.MatmulPerfMode.DoubleRow

N = 50000
E = 500000
DIN = 256
DH = 512
NG = 64
DOUT = 16

NCORES = 8
SLICE = N // NCORES  # 6250
WW = 64  # dst window width (one-hot width)
NW = (SLICE + WW - 1) // WW  # 98 windows
NPAD = 6272  # 49 * 128 node columns
NCHK = NPAD // 128  # 49 node chunks
NGRP = 13  # 12 groups of 512 node cols + 1 of 128

# tuning knobs
USE_DR_DENSE = True  # fp8 DoubleRow for the W1 dense
OH_ON_GPSIMD = False  # Pool engine lacks is_equal (walrus ISA check)

_COMPILED: dict = {}


def _group_info(g):
    """(first window, #windows, node col0, #node cols, first chunk, #chunks)"""
    if g < 12:
        return (8 * g, 8, 512 * g, 512, 4 * g, 4)
    return (96, 2, 6144, 128, 48, 1)


def _layout(K1):
    """Batches of <=4 windows: [(g, ws, {w: [(gcol, grel)]}, nch, c0)]."""
    batches = []
    gcol = 0
    for g in range(NGRP):
        w0, nwin, _, _, _, _ = _group_info(g)
        nhalf = 2 if nwin == 8 else 1
        for half in range(nhalf):
            ws = list(range(w0 + half * 4, min(w0 + (half + 1) * 4, w0 + nwin)))
            c0 = gcol
            rel = 0
            wch = {}
            for w in ws:
                lst = []
                for _ in range(int(K1[w])):
                    lst.append((gcol, rel))
                    gcol += 1
                    rel += 1
                wch[w] = lst
            batches.append((g, ws, wch, rel, c0))
    return batches, gcol


def _preprocess(x, edge_index, batch):
    src = np.asarray(edge_index[0], dtype=np.int64)
    dst = np.asarray(edge_index[1], dtype=np.int64)
    batch = np.asarray(batch, dtype=np.int64)

    deg = np.bincount(dst, minlength=N).astype(np.float64) + 1.0
    dinv = (1.0 / np.sqrt(deg)).astype(np.float32)
    cnt = np.maximum(np.bincount(batch, minlength=NG), 1)

    loops = np.arange(N, dtype=np.int64)

    # ---------- conv1: edges + self-loops grouped by (core, 64-window) ----------
    s1 = np.concatenate([src, loops])
    d1 = np.concatenate([dst, loops])
    norm1 = dinv[s1] * dinv[d1]
    core1 = d1 // SLICE
    win1 = (d1 % SLICE) // WW
    key1 = core1 * NW + win1
    order1 = np.argsort(key1, kind="stable")
    ss1, ds1, nn1 = s1[order1], d1[order1], norm1[order1]
    counts1 = np.bincount(key1, minlength=NCORES * NW).reshape(NCORES, NW)
    starts1 = np.zeros(NCORES * NW + 1, dtype=np.int64)
    np.cumsum(counts1.reshape(-1), out=starts1[1:])
    K1 = np.ceil(counts1.max(axis=0) / 128).astype(np.int64)  # [NW]

    meta = tuple(int(v) for v in K1)
    batches, C1 = _layout(K1)

    # ---------- fused conv2+pool coefficient matrix A[s, G] ----------
    gd = batch[dst]
    A = np.bincount(src * NG + gd, weights=dinv[dst].astype(np.float64),
                    minlength=N * NG).reshape(N, NG).astype(np.float32)
    A[loops, batch] += dinv
    A *= dinv[:, None]
    # bake the mean-pool 1/cnt into A, x64 to keep fp8 AR payloads in range;
    # the tail activation divides by 64 via its scale parameter
    A *= (64.0 / np.maximum(cnt, 1).astype(np.float32))[None, :]

    xf = np.asarray(x, np.float32)

    per_core = []
    for c in range(NCORES):
        src_cols = np.zeros((C1, 128), dtype=np.int64)
        nrm_cols = np.zeros((C1, 128), dtype=np.float32)
        dst_cols = np.full((C1, 128), -1.0, dtype=np.float32)
        for _g, ws, wch, _nch, _c0 in batches:
            for w in ws:
                gi = c * NW + w
                e0, e1 = starts1[gi], starts1[gi + 1]
                n_e = int(e1 - e0)
                cols = wch[w]
                k = len(cols)
                sv = np.zeros(k * 128, dtype=np.int64)
                sv[:n_e] = ss1[e0:e1]
                nv = np.zeros(k * 128, dtype=np.float32)
                nv[:n_e] = nn1[e0:e1]
                dv = np.full(k * 128, -1.0, dtype=np.float32)
                dv[:n_e] = (ds1[e0:e1] - (c * SLICE + w * WW)).astype(np.float32)
                for j, (gcol, _r) in enumerate(cols):
                    src_cols[gcol] = sv[j * 128 : (j + 1) * 128]
                    nrm_cols[gcol] = nv[j * 128 : (j + 1) * 128]
                    dst_cols[gcol] = dv[j * 128 : (j + 1) * 128]
        rows = xf[src_cols.reshape(-1)] * nrm_cols.reshape(-1)[:, None]
        x_edges = np.ascontiguousarray(
            rows.astype(ml_dtypes.float8_e4m3).reshape(C1, 128, DIN).transpose(1, 0, 2)
        ).reshape(128, C1 * DIN)

        Ac = np.zeros((NPAD, NG), dtype=np.float32)
        Ac[:SLICE] = A[c * SLICE : (c + 1) * SLICE]
        a_sb = np.ascontiguousarray(
            Ac.reshape(NCHK, 128, NG).transpose(1, 0, 2)
        ).reshape(128, NCHK * NG).astype(ml_dtypes.bfloat16)

        iota_cols = np.tile(np.arange(WW, dtype=np.float32)[None, :], (128, 1))
        per_core.append(
            dict(
                x_edges=x_edges,
                dst1=np.ascontiguousarray(
                    np.concatenate([dst_cols.T, iota_cols], axis=1)
                ).astype(ml_dtypes.bfloat16),
                a_mat=a_sb,
            )
        )
    return meta, per_core, cnt.astype(np.float32)


def _build_program(meta):
    K1 = np.array(meta)
    batches, C1 = _layout(K1)
    max_nch = max(b[3] for b in batches)

    nc = bacc.Bacc("TRN2", target_bir_lowering=False, debug=False, num_devices=NCORES)

    def din(name, shape, dt=F32):
        return nc.dram_tensor(name, shape, dt, kind="ExternalInput").ap()

    x_edges = din("x_edges", [128, C1 * DIN], F8)
    dst1 = din("dst1", [128, C1 + WW], BF16)  # last WW cols: iota 0..63
    a_mat = din("a_mat", [128, NCHK * NG], BF16)
    w1dr = din("w1dr", [128, 2 * DH], F8)  # [p, k(2), m(4), 128] fp8 pairs
    w1bf = din("w1bf", [128, 2 * DH], BF16)  # [p, k(2), fo(512)] bf16 fallback
    # merged bf16 consts: [0:1024] w2b, [1024:1280] wf1, [1280:1296] wf2
    wc_bf = din("wc_bf", [128, 1296], BF16)
    # merged f32 consts: [0:4] b1c, [4:6] b2h, [6:7] bf1c, [7:8] bf2c (rows<16)
    fc32 = din("fc32", [128, 8])
    out = nc.dram_tensor("out", [DOUT, NG], F32, kind="ExternalOutput").ap()

    with tile.TileContext(nc) as tc:
        with (
            tc.tile_pool(name="const", bufs=1) as cp,
            tc.tile_pool(name="big", bufs=1) as bigp,
            tc.tile_pool(name="work", bufs=1) as wp,
            tc.tile_pool(name="psum", bufs=1, space="PSUM") as pp,
            tc.tile_pool(name="dram", bufs=1, space="DRAM") as dp,
        ):
            def load(ap_in, shape, dt=F32, pool=cp):
                t = pool.tile(shape, dt, name=ap_in.tensor.name + "_sb")
                nc.sync.dma_start(t[:], ap_in[:])
                return t

            # loads gating the pipeline start go first; the rest after batch 0
            dst1_sb = load(dst1, [128, C1 + WW], BF16)

            h1s = [bigp.tile([128, NPAD], BF16, name=f"h1s_{k}") for k in range(4)]

            sfg_groups: dict = {}

            def sfg_of(g):
                # fp8 feature-major conv1 aggregation for group g: [p, k(2), n]
                if g not in sfg_groups:
                    sfg_groups[g] = wp.tile(
                        [128, 2, 512], F8, tag="sfg", bufs=2, name=f"sfg_{g}"
                    )
                return sfg_groups[g]

            # persistent feature-major pool partials: pgo[h] = [128 o, 64 G]
            pgo = [pp.tile([128, NG], F32, name=f"pgo_{h}") for h in range(2)]
            g_local = dp.tile([128, 2 * NG], F8, name="gl")
            g_ag = dp.tile([NCORES * 128, 2 * NG], F8, addr_space="Shared", name="gag")
            gs_all = wp.tile([128, NCORES, 2 * NG], F8, name="gs_all")
            red = wp.tile([128, 2 * NG], F32, name="red")

            def emit_allgather():
                # AllGather the 8 fp8 pool partials (cheaper than AllReduce:
                # no reduce phase on the CC cores) and sum them on the DVE.
                gsb = wp.tile([128, 2, NG], F8, name="gsb")
                for h in range(2):
                    nc.vector.tensor_copy(gsb[:, h, :], pgo[h][:])
                nc.sync.dma_start(g_local[:], gsb[:].rearrange("p h g -> p (h g)"))
                nc.gpsimd.collective_compute(
                    "AllGather",
                    OP.bypass,
                    replica_groups=[list(range(NCORES))],
                    ins=[g_local.opt()],
                    outs=[g_ag.opt()],
                )
                nc.sync.dma_start(
                    gs_all[:],
                    g_ag[:].rearrange("(r p) c -> p r c", r=NCORES),
                )
                nc.vector.tensor_reduce(
                    out=red[:],
                    in_=gs_all[:].rearrange("p r c -> p c r"),
                    axis=mybir.AxisListType.X,
                    op=OP.add,
                )

            def emit_stream(bi):
                """One G1 DMA + one-hot build covering one batch (<=4 windows)."""
                _g, _ws, _wch, nch, c0 = batches[bi]
                G1 = wp.tile([128, nch, DIN], F8, tag="G1", bufs=3, name=f"g1b_{bi}")
                nc.sync.dma_start(
                    G1[:].rearrange("p c d -> p (c d)"),
                    x_edges[:, c0 * DIN : (c0 + nch) * DIN],
                )
                oh = wp.tile([128, nch, WW], BF16, tag="oh", bufs=3, name=f"ohb_{bi}")
                eng = nc.gpsimd if (OH_ON_GPSIMD and bi % 2 == 1) else nc.vector
                eng.tensor_tensor(
                    out=oh[:],
                    in0=dst1_sb[:, C1 : C1 + WW]
                    .rearrange("p (c o) -> p c o", c=1)
                    .to_broadcast([128, nch, WW]),
                    in1=dst1_sb[:, c0 : c0 + nch]
                    .rearrange("p (c o) -> p c o", o=1)
                    .to_broadcast([128, nch, WW]),
                    op=OP.is_equal,
                )
                return G1, oh

            def emit_batch(g, ws, wch, nch, c0, G1, oh):
                sfg = sfg_of(g)
                nw = len(ws)
                wb0 = ws[0] - 8 * g
                pa = pp.tile([128, nw, 2, WW], F32, tag="agg", bufs=2, name=f"pa_{ws[0]}")
                for w in ws:
                    cols = wch[w]
                    wrel = w - ws[0]
                    for j, (_gcol, grel) in enumerate(cols):
                        for h in range(2):
                            nc.tensor.matmul(
                                out=pa[:, wrel, h, :],
                                lhsT=G1[:, grel, h * 128 : (h + 1) * 128],
                                rhs=oh[:, grel, :],
                                start=(j == 0),
                                stop=(j == len(cols) - 1),
                            )
                for h in range(2):
                    nc.scalar.activation(
                        sfg[:, h, wb0 * WW : (wb0 + nw) * WW],
                        pa[:, :, h, :],
                        AF.Copy,
                    )

            def emit_dense(g):
                _, _, n0, ncols, _, _ = _group_info(g)
                sfg = sfg_of(g)
                for m in range(4):
                    ph = pp.tile([128, 512], F32, tag="h1", bufs=2, name=f"ph_{g}_{m}")
                    if USE_DR_DENSE:
                        nc.tensor.matmul(
                            out=ph[:, :ncols],
                            lhsT=w1_sb[:, :, m, :],
                            rhs=sfg[:, :, :ncols],
                            start=True,
                            stop=True,
                            perf_mode=DR,
                        )
                    else:
                        for k in range(2):
                            nc.tensor.matmul(
                                out=ph[:, :ncols],
                                lhsT=w1f_sb[:, k, m * 128 : (m + 1) * 128],
                                rhs=sfg[:, k, :ncols],
                                start=(k == 0),
                                stop=(k == 1),
                            )
                    nc.scalar.activation(
                        h1s[m][:, n0 : n0 + ncols], ph[:, :ncols], AF.Relu,
                        bias=fc_sb[:, m : m + 1],
                    )

            def emit_pA(cc):
                c0 = cc * 128
                ppm = pp.tile([128, DH // 2], F32, tag="p2", bufs=2, name=f"ppm_{cc}")
                for k in range(4):
                    nc.tensor.matmul(
                        out=ppm[:],
                        lhsT=h1s[k][:, c0 : c0 + 128],
                        rhs=wc_sb[:, k * (DH // 2) : (k + 1) * (DH // 2)],
                        start=(k == 0),
                        stop=(k == 3),
                    )
                pb = wp.tile([128, DH // 2], BF16, tag="pb", bufs=2, name=f"pb_{cc}")
                nc.vector.tensor_copy(pb[:], ppm[:])
                for h in range(2):
                    nc.tensor.matmul(
                        out=pgo[h][:],
                        lhsT=pb[:, h * 128 : (h + 1) * 128],
                        rhs=a_sb[:, cc * NG : (cc + 1) * NG],
                        start=(cc == 0),
                        stop=(cc == NCHK - 1),
                    )

            streams = {0: emit_stream(0), 1: emit_stream(1)}
            if USE_DR_DENSE:
                w1_sb = load(w1dr, [128, 2, 4, 128], F8)
            else:
                w1f_sb = load(w1bf, [128, 2, DH], BF16)
            fc_sb = load(fc32, [128, 8])
            # warm-up collectives: absorb the CC barrier + cold firmware setup
            # during the main phase so the real AllGather launches warm
            warm_l = dp.tile([128, 128], F8, name="warm_l")
            warm_r = dp.tile([128, 128], F8, addr_space="Shared", name="warm_r")
            warm_ag = dp.tile([128 * 8, 128], F8, addr_space="Shared", name="warm_ag")
            nc.gpsimd.collective_compute(
                "AllReduce", OP.add, replica_groups=[list(range(NCORES))],
                ins=[warm_l.opt()], outs=[warm_r.opt()],
            )
            nc.gpsimd.collective_compute(
                "AllGather", OP.bypass, replica_groups=[list(range(NCORES))],
                ins=[warm_l.opt()], outs=[warm_ag.opt()],
            )
            bidx = 0
            for g in range(NGRP):
                _, nwin, _, _, cc0, nccs = _group_info(g)
                nb = 2 if nwin == 8 else 1
                for _b in range(nb):
                    if bidx + 2 < len(batches):
                        streams[bidx + 2] = emit_stream(bidx + 2)
                    G1, oh = streams.pop(bidx)
                    emit_batch(*batches[bidx], G1, oh)
                    bidx += 1
                    if bidx == 1:
                        a_sb = load(a_mat, [128, NCHK * NG], BF16)
                        wc_sb = load(wc_bf, [128, 1296], BF16)
                emit_dense(g)
                for cc in range(cc0, cc0 + nccs):
                    emit_pA(cc)

            # ---- tail: AllGather + DVE reduce + mean/bias/relu + MLP,
            # all feature-major (1/cnt baked into A on host, x64; /64 here)
            emit_allgather()
            curv = red[:].rearrange("p (h g) -> p h g", h=2)
            grelu = wp.tile([128, 2, NG], BF16, name="grelu")
            for h in range(2):
                nc.scalar.activation(
                    grelu[:, h, :], curv[:, h, :], AF.Relu,
                    bias=fc_sb[:, 4 + h : 5 + h], scale=1.0 / 64.0,
                )
            pz = pp.tile([128, NG], F32, tag="p2", bufs=2, name="pz")
            for k in range(2):
                nc.tensor.matmul(
                    out=pz[:],
                    lhsT=wc_sb[:, 1024 + k * 128 : 1024 + (k + 1) * 128],
                    rhs=grelu[:, k, :],
                    start=(k == 0),
                    stop=(k == 1),
                )
            zsb = wp.tile([128, NG], BF16, name="zsb")
            nc.scalar.activation(zsb[:], pz[:], AF.Relu, bias=fc_sb[:, 6:7])
            po = pp.tile([DOUT, NG], F32, tag="agg", bufs=2, name="po")
            nc.tensor.matmul(
                out=po[:], lhsT=wc_sb[:, 1280:1296], rhs=zsb[:], start=True, stop=True
            )
            osb = wp.tile([DOUT, NG], F32, name="osb")
            nc.scalar.activation(osb[:], po[:], AF.Relu, bias=fc_sb[:16, 7:8])
            nc.sync.dma_start(out[:], osb[:])

    nc.compile()
    return nc


def _get_program(meta):
    if meta not in _COMPILED:
        _COMPILED[meta] = _build_program(meta)
    return _COMPILED[meta]


def _make_in_maps(W1, b1, W2, b2, Wf1, bf1, Wf2, bf2, per_core, cnt, meta):
    bf = ml_dtypes.bfloat16
    f8 = ml_dtypes.float8_e4m3
    W1 = np.asarray(W1, np.float32)
    W2 = np.asarray(W2, np.float32)
    Wf1 = np.asarray(Wf1, np.float32)
    b2 = np.asarray(b2, np.float32)
    K1 = np.array(meta)
    batches, _C1 = _layout(K1)
    max_nch = max(b[3] for b in batches)

    # w1dr[p, k, m, c] = W1[k*128+p, m*128+c]
    w1dr = np.ascontiguousarray(
        W1.reshape(2, 128, 4, 128).transpose(1, 0, 2, 3).reshape(128, 2 * DH)
    )
    w1bf = np.ascontiguousarray(
        W1.reshape(2, 128, DH).transpose(1, 0, 2).reshape(128, 2 * DH)
    )
    w2b = np.ascontiguousarray(
        np.concatenate([W2[k * 128 : (k + 1) * 128, :] for k in range(4)], axis=1)
    )
    wf1b = np.ascontiguousarray(
        Wf1.reshape(2, 128, DH // 4).transpose(1, 0, 2).reshape(128, 2 * (DH // 4))
    )
    wc = np.concatenate([w2b, wf1b, np.asarray(Wf2, np.float32)], axis=1)
    fc = np.zeros((128, 8), np.float32)
    fc[:, 0:4] = np.asarray(b1, np.float32).reshape(DH // 128, 128).T
    fc[:, 4:6] = b2.reshape(2, 128).T
    fc[:, 6] = np.asarray(bf1, np.float32).reshape(DH // 4)
    fc[:DOUT, 7] = np.asarray(bf2, np.float32).reshape(DOUT)
    shared = dict(
        w1dr=w1dr.astype(f8),
        w1bf=w1bf.astype(bf),
        wc_bf=np.ascontiguousarray(wc).astype(bf),
        fc32=fc,
    )
    return [dict(shared, **per_core[c]) for c in range(NCORES)]


def kernel(
    x, W1, b1, W2, b2, Wf1, bf1, Wf2, bf2, edge_index, batch, num_graphs, _trace=False
):
    assert int(num_graphs) == NG
    meta, per_core, cnt = _preprocess(
        np.asarray(x), np.asarray(edge_index), np.asarray(batch)
    )
    nc = _get_program(meta)
    in_maps = _make_in_maps(W1, b1, W2, b2, Wf1, bf1, Wf2, bf2, per_core, cnt, meta)
    res = bass_utils.run_bass_kernel_spmd(
        nc, in_maps, core_ids=list(range(NCORES)), trace=_trace
    )
    out = np.ascontiguousarray(np.asarray(res.results[0]["out"], np.float32).T)
    if _trace:
        kernel._last_results = res
    return out
